# revision 7
# baseline (speedup 1.0000x reference)
"""GNN message-passing kernel for 8 TRN2 NeuronCores (Bass/Tile), v2.

Design (edge-parallel by destination, per sharding hint):
  - Nodes sharded into 8 contiguous ranges; within each shard nodes are
    relabeled by descending local in-degree so the k-th incoming edge of
    every node forms a tile whose destinations are a prefix [0, n_k).
  - The edge MLP input concat is decomposed: pre = A[row] + B[col] + C with
    A = h@We1[:H], B = h@We1[H:2H], C = ef@We1[2H:].  A is read
    sequentially (prefix property), B is gathered per 128-edge tile via
    indirect DMA from a replicated bf16 HBM table, C is computed on-chip
    from a streamed bf16 edge-feature tensor (poison row zeroes pad edges
    through silu).
  - Tiles are processed in PAIRS (k, j),(k+1, j) stacked on partitions
    0-63 / 64-127; both halves target the same t1 node columns so one
    matmul per pair handles phi_edge L2 (block-diag We2), and one matmul
    per pair both applies Wn1[H:] per-edge and scatter-adds into the
    per-band PSUM accumulator t1 (segment-sum fused into the node MLP).
  - B tables are AllGathered per *band* so each layer's collective
    overlaps the previous layer's compute; layer 0's table is computed
    locally from node_features on every core (no collective).
  - Activations/weights in bf16 (tolerance 2e-2), PSUM accumulation fp32.
"""

import sys

if "/opt/trn_rl_repo" not in sys.path:
    sys.path.insert(0, "/opt/trn_rl_repo")

import numpy as np
import ml_dtypes

import concourse.bass as bass
import concourse.mybir as mybir
import concourse.tile as tile
from concourse.bass import IndirectOffsetOnAxis
from concourse.bass_utils import run_bass_kernel_spmd

NCORES = 8
P = 128
BATCH = 4          # pairs per edge batch (= 8 original tiles)
BAND = 1536        # nodes per t1 accumulation band (3 PSUM banks)
POISON = -100.0
DEBUG = False
F32 = mybir.dt.float32
BF16 = mybir.dt.bfloat16
I32 = mybir.dt.int32
ACT = mybir.ActivationFunctionType
BF = ml_dtypes.bfloat16


def _spread_swdge_queues(nc, nq=2):
    """indirect_dma_start pins queue="qPoolDynamic" (queue 0); alternate
    gathers across the allocated SWDGE queues so both GpSimd cores emit
    descriptors in parallel."""
    i = 0
    for func in nc.m.functions:
        for bb in func.blocks:
            for inst in bb.instructions:
                if (isinstance(inst, mybir.InstDMACopy)
                        and getattr(inst, "queue", None) == "qPoolDynamic"
                        and any(getattr(a, "dynamic_ap_info", None) is not None
                                for a in inst.ins + inst.outs)):
                    q = i % nq
                    if q:
                        inst.queue = f"qPoolDynamic{q}"
                    i += 1
    return i


def _split_sync_waits(nc):
    """Two walrus/HW quirks, one pass:
    1. this walrus build accepts only one sync-wait per instruction; move
       extras onto same-engine NOPs inserted just before.
    2. walrus lowers a self-loading InstMatmult into LDWEIGHTS+MATMUL with
       the sync-wait on the MATMUL only, so the stationary-operand load can
       race ahead of its producer; move ALL matmul waits onto NOPs so the
       sequencer stalls before LDWEIGHTS."""
    cnt = 0
    for func in nc.m.functions:
        for bb in func.blocks:
            out = []
            changed = False
            for inst in bb.instructions:
                si = inst.sync_info
                if si is not None and si.on_wait:
                    is_mm = isinstance(inst, mybir.InstMatmult)
                    if is_mm or len(si.on_wait) > 1:
                        extra = list(si.on_wait if is_mm else si.on_wait[:-1])
                        keep = None if is_mm else si.on_wait[-1]
                        del si.on_wait[:]
                        if keep is not None:
                            si.on_wait.append(keep)
                        for w in extra:
                            cnt += 1
                            nop = mybir.InstNoOp(name=f"WS-{cnt}", ins=[],
                                                 outs=[])
                            nop.engine = inst.engine
                            nop.sync_info = mybir.SyncInfo(on_wait=[w],
                                                           on_update=[])
                            out.append(nop)
                            changed = True
                out.append(inst)
            if changed:
                bb.instructions[:] = out
    return cnt


# ---------------------------------------------------------------- host prep

def _prep(node_features, edge_indices, edges_features, We1):
    N = node_features.shape[0]
    E = edge_indices.shape[1]
    ED = edges_features.shape[1]
    row = edge_indices[0].astype(np.int64)
    col = edge_indices[1].astype(np.int64)
    SH = -(-N // NCORES)
    SHP = -(-SH // P) * P
    NB = -(-SHP // BAND)
    band_rows = [min(BAND, SHP - b * BAND) for b in range(NB)]
    band_base = [b * BAND for b in range(NB)]

    # per-shard degree sort
    perms, inv_perms, deg_sorted = [], [], []
    core_edges = []
    for s in range(NCORES):
        lo, hi = s * SH, min((s + 1) * SH, N)
        eidx = np.nonzero((row >= lo) & (row < hi))[0]
        r_loc = row[eidx] - lo
        nloc = hi - lo
        deg = np.bincount(r_loc, minlength=nloc)
        perm = np.argsort(-deg, kind="stable")
        inv = np.empty(nloc, dtype=np.int64)
        inv[perm] = np.arange(nloc)
        slot = inv[r_loc]
        order = np.lexsort((col[eidx], slot))
        core_edges.append((slot[order], col[eidx][order], eidx[order]))
        perms.append(perm)
        inv_perms.append(inv)
        deg_sorted.append(deg[perm])

    maxdeg = max((int(d[0]) if len(d) else 0) for d in deg_sorted)
    n_k = [max(int((d > k).sum()) for d in deg_sorted) for k in range(maxdeg)]
    KP = -(-maxdeg // 2)

    # pair grid: band-major, then kp, then j; pad per band to BATCH
    pair_of = {}
    pairs = []          # (band, jb) per pair
    for b in range(NB):
        blo, bhi = band_base[b], band_base[b] + band_rows[b]
        for kp in range(KP):
            hi = min(n_k[2 * kp], bhi)
            j = blo
            while j < hi:
                pair_of[(kp, j)] = len(pairs)
                pairs.append((b, j))
                j += P
        while len(pairs) % BATCH:
            pairs.append((b, blo))          # dummy pair: all-pad
    NPB = len(pairs)

    # btab row index for a global node id: band-major, within a (band,
    # shard) block LANE-major (row = lane*nch + chunk) so the per-band
    # Bsh write is a single DMA from a [128, nch*H] SBUF tile.
    col_btab = np.empty(N, dtype=np.int64)
    br = np.array(band_rows)
    bb8 = np.array([band_base[bb] * NCORES for bb in range(NB)])
    nchs = br // P
    for s in range(NCORES):
        lo, hi = s * SH, min((s + 1) * SH, N)
        loc = inv_perms[s]                   # local node -> slot
        b2 = loc // BAND
        r2 = loc % BAND
        col_btab[lo:hi] = (bb8[b2] + s * br[b2]
                           + (r2 % P) * nchs[b2] + r2 // P)

    # per-shard streams
    efts, coffs_all = [], []
    for s in range(NCORES):
        slot, c_g, eidx = core_edges[s]
        first = np.searchsorted(slot, slot, side="left")
        rank = np.arange(len(slot)) - first
        keep = rank < maxdeg
        slot, c_g, eidx, rank = slot[keep], c_g[keep], eidx[keep], rank[keep]
        kp = rank // 2
        half = rank % 2
        jb = (slot // P) * P
        pidx = np.array([pair_of[(k, j)] for k, j in zip(kp, jb)],
                        dtype=np.int64)
        lane = slot % P

        eft = np.zeros((2 * (ED + 1), NPB * P), np.float32)
        eft[ED, :] = 1.0                     # pad indicator, half 0
        eft[2 * ED + 1, :] = 1.0             # pad indicator, half 1
        epos = pidx * P + lane
        # scatter: eft[half*(ED+1) + d, epos] = ef[eidx, d]; indicator -> 0
        ef_s = edges_features[eidx]
        for hval in (0, 1):
            m = half == hval
            base = hval * (ED + 1)
            eft[base + ED, epos[m]] = 0.0
            for d in range(ED):
                eft[base + d, epos[m]] = ef_s[m, d]

        co = np.zeros((P, 2 * NPB), np.int32)
        co[lane, 2 * pidx + half] = col_btab[c_g]
        efts.append(eft.astype(BF))
        coffs_all.append(co)

    # halves empty on ALL shards (SPMD shares one instruction stream):
    # their gathers fetch garbage that silu discards -> skip them.
    nk_per_s = [[int((deg_sorted[s] > k).sum()) for k in range(maxdeg)]
                for s in range(NCORES)]
    half_used = np.zeros((NPB, 2), bool)
    for p, (b, jb) in enumerate(pairs):
        kp_list = [k for (k, j2) in pair_of.items() if False]  # unused
    for (kp2, j2), p in pair_of.items():
        for h in range(2):
            k = 2 * kp2 + h
            if k < maxdeg and any(nk_per_s[s][k] > j2 for s in range(NCORES)):
                half_used[p, h] = True

    # pad-half count per slot (for the be2 pad-message correction): every
    # (pair, half) covering a slot that has no real edge contributes a
    # silu(be2) message; cancel it via the t1-init matmul.
    halves_per_block = np.zeros(SHP // P, np.int64)
    for (_, jb) in pairs:
        halves_per_block[jb // P] += 2
    npads = []
    for s in range(NCORES):
        d = np.zeros(SHP, np.int64)
        ds = deg_sorted[s]
        d[:len(ds)] = ds
        np_s = halves_per_block[(np.arange(SHP) // P)] - d
        npads.append((-np_s.astype(np.float32))[None, :])

    return dict(SH=SH, SHP=SHP, NB=NB, band_rows=band_rows,
                band_base=band_base, NPB=NPB, pairs=pairs, KP=KP,
                perms=perms, inv_perms=inv_perms, efts=efts,
                coffs=coffs_all, maxdeg=maxdeg, E=E, ED=ED, npads=npads,
                half_used=half_used)


def _blob_layout(L, H, ND, ED, OD):
    lay, o = {}, 0

    def add(name, w):
        nonlocal o
        lay[name] = (o, w)
        o += w

    add("I128", P)
    add("I64", H)
    add("embW", H)           # [ND+1, H] rows (emb_b folded)
    add("Wu1", H)
    add("Wu2", OD)           # [H+1, OD] rows (bu2 folded)
    for l in range(L):
        add(f"Wefd{l}", P)   # [2*(ED+1), 128] block-diag ef weights + poison
        add(f"BD{l}", P)     # [128, 128] block-diag We2
        add(f"Wn1a2{l}", H)  # [128, 64] stacked Wn1[H:]
        add(f"We1rd{l}", P)  # [64, 128] duplicated We1[:H]
        add(f"We1c{l}", H)   # [64, 64] We1[H:2H]
        add(f"Wn1h{l}", H)   # [64, 64] Wn1[:H]
        add(f"Wn2{l}", H)    # [65, 64] Wn2 + bn2 row
    add("nfW0", H)           # [ND+1, H] (embW+emb_b) @ We1c[0] for local B0
    lay["total"] = o
    return lay


def _bias_layout(L, H):
    lay, o = {}, 0

    def add(name, w):
        nonlocal o
        lay[name] = (o, w)
        o += w

    for l in range(L):
        add(f"be1st{l}", 1)   # [128, 1]
        add(f"be2st{l}", 1)   # [128, 1]
        add(f"bn1{l}", 1)     # [64, 1]
    add("bu1", 1)             # [64, 1]
    lay["total"] = o
    return lay


def _blob32_layout(L, H, ND, OD):
    """f32 weights for matmuls whose other operand is f32 (hT / m2 / nf)."""
    lay, o = {}, 0

    def add(name, w):
        nonlocal o
        lay[name] = (o, w)
        o += w

    add("I64", H)
    add("embW", H)            # [ND+1, H]
    add("Wu1", H)
    for l in range(L):
        add(f"Wn1h{l}", H)    # [65, 64]: row 64 = Wn1a.T @ silu(be2) (pad fix)
        add(f"We1rd{l}", P)   # [64, 128]
        add(f"We1c{l}", H)    # [64, 64]
        add(f"Wn1a2{l}", H)   # [128, 64]
    lay["total"] = o
    return lay


def _weights_blobs(emb_W, emb_b, We1, be1, We2, be2, Wn1, bn1, Wn2, bn2,
                   Wu1, bu1, Wu2, bu2, H, ND, ED, OD):
    L = We1.shape[0]
    lay = _blob_layout(L, H, ND, ED, OD)
    blob = np.zeros((P, lay["total"]), np.float32)

    def put(name, arr, prow=0):
        o, _ = lay[name]
        blob[prow:prow + arr.shape[0], o:o + arr.shape[1]] = arr

    put("I128", np.eye(P, dtype=np.float32))
    put("I64", np.eye(H, dtype=np.float32))
    put("embW", np.vstack([emb_W, emb_b[None, :]]))
    put("Wu1", Wu1)
    put("Wu2", np.vstack([Wu2, bu2[None, :]]))
    for l in range(L):
        wef = np.vstack([We1[l][2 * H:], np.full((1, H), POISON, np.float32)])
        wefd = np.zeros((2 * (ED + 1), P), np.float32)
        wefd[:ED + 1, :H] = wef
        wefd[ED + 1:, H:] = wef
        put(f"Wefd{l}", wefd)
        bd = np.zeros((P, P), np.float32)
        bd[:H, :H] = We2[l]
        bd[H:, H:] = We2[l]
        put(f"BD{l}", bd)
        put(f"Wn1a2{l}", np.vstack([Wn1[l][H:], Wn1[l][H:]]))
        put(f"We1rd{l}", np.hstack([We1[l][:H], We1[l][:H]]))
        put(f"We1c{l}", We1[l][H:2 * H])
        sb2 = be2[l] / (1.0 + np.exp(-be2[l]))
        put(f"Wn1h{l}", np.vstack([Wn1[l][:H], (Wn1[l][H:].T @ sb2)[None, :]]))
        put(f"Wn2{l}", np.vstack([Wn2[l], bn2[l][None, :]]))
    put("nfW0", np.vstack([emb_W, emb_b[None, :]]) @ We1[0][H:2 * H])

    blay = _bias_layout(L, H)
    bias = np.zeros((P, blay["total"]), np.float32)

    def putb(name, arr):
        o, _ = blay[name]
        bias[:arr.shape[0], o:o + 1] = arr[:, None]

    for l in range(L):
        putb(f"be1st{l}", np.concatenate([be1[l], be1[l]]))
        putb(f"be2st{l}", np.concatenate([be2[l], be2[l]]))
        putb(f"bn1{l}", bn1[l])
    putb("bu1", bu1)

    lay32 = _blob32_layout(L, H, ND, OD)
    blob32 = np.zeros((P, lay32["total"]), np.float32)

    def put32(name, arr):
        o, _ = lay32[name]
        blob32[:arr.shape[0], o:o + arr.shape[1]] = arr

    put32("I64", np.eye(H, dtype=np.float32))
    put32("embW", np.vstack([emb_W, emb_b[None, :]]))
    put32("Wu1", Wu1)
    for l in range(L):
        sb2 = be2[l] / (1.0 + np.exp(-be2[l]))       # silu(be2)
        corr = (Wn1[l][H:].T @ sb2)[None, :]          # pad-message row
        put32(f"Wn1h{l}", np.vstack([Wn1[l][:H], corr]))
        put32(f"We1rd{l}", np.hstack([We1[l][:H], We1[l][:H]]))
        put32(f"We1c{l}", We1[l][H:2 * H])
        put32(f"Wn1a2{l}", np.vstack([Wn1[l][H:], Wn1[l][H:]]))
    return blob.astype(BF), bias, blob32


# ---------------------------------------------------------------- builder

def _build(ND, ED, L, H, OD, meta, wcols, bcols, w32cols):
    lay = _blob_layout(L, H, ND, ED, OD)
    blay = _bias_layout(L, H)
    lay32 = _blob32_layout(L, H, ND, OD)
    SHP, NB = meta["SHP"], meta["NB"]
    band_rows, band_base = meta["band_rows"], meta["band_base"]
    NPB, pairs = meta["NPB"], meta["pairs"]
    EFR = 2 * (ED + 1)      # eft rows
    TROWS = SHP * NCORES    # btab rows

    nc = bass.Bass("TRN2", num_devices=NCORES, num_swdge_queues=2,
                   dynamic_dma_scratch_size=65536)
    nfT_d = nc.dram_tensor("nfT", [ND + 1, SHP], BF16, kind="ExternalInput")
    w32_d = nc.dram_tensor("w32", [P, w32cols], F32, kind="ExternalInput")
    dbg = {}
    if DEBUG:
        dbg["h0"] = nc.dram_tensor("dbg_h0", [H, SHP], F32,
                                   kind="ExternalOutput")
        dbg["at0"] = nc.dram_tensor("dbg_at0", [P, SHP], F32,
                                    kind="ExternalOutput")
        dbg["bt0"] = nc.dram_tensor("dbg_bt0", [SHP * NCORES, H], F32,
                                    kind="ExternalOutput")
        dbg["rt0"] = nc.dram_tensor("dbg_rt0", [H, BAND], F32,
                                    kind="ExternalOutput")
        dbg["h1"] = nc.dram_tensor("dbg_h1", [H, SHP], F32,
                                   kind="ExternalOutput")
        dbg["m2"] = nc.dram_tensor("dbg_m2", [P, NPB * P], F32,
                                   kind="ExternalOutput")
        dbg["pre"] = nc.dram_tensor("dbg_pre", [P, NPB * P], F32,
                                    kind="ExternalOutput")
        dbg["bg"] = nc.dram_tensor("dbg_bg", [P, NPB * P], F32,
                                   kind="ExternalOutput")
    nfTg_d = nc.dram_tensor("nfTg", [ND + 1, TROWS], BF16,
                            kind="ExternalInput")     # global, btab order
    eft_d = nc.dram_tensor("eft", [EFR, NPB * P], BF16, kind="ExternalInput")
    coffs_d = nc.dram_tensor("coffs", [P, 2 * NPB], I32, kind="ExternalInput")
    wb_d = nc.dram_tensor("wblob", [P, wcols], BF16, kind="ExternalInput")
    bb_d = nc.dram_tensor("bblob", [P, bcols], F32, kind="ExternalInput")
    npads_d = nc.dram_tensor("npads", [1, SHP], BF16,
                             kind="ExternalInput")
    out_d = nc.dram_tensor("out", [OD, SHP], F32, kind="ExternalOutput")
    btabs = [nc.dram_tensor(f"btab{l}", [TROWS, H], BF16, kind="Internal",
                            addr_space="Shared") for l in range(L)]
    bshs = [nc.dram_tensor(f"bsh{l}", [SHP, H], BF16, kind="Internal")
            for l in range(1, L)]

    with tile.TileContext(nc) as tc:
        with tc.tile_pool(name="const", bufs=1) as cp, \
             tc.tile_pool(name="st", bufs=4) as st, \
             tc.tile_pool(name="ppre", bufs=2, space="PSUM") as ppre, \
             tc.tile_pool(name="pz", bufs=1, space="PSUM") as pz, \
             tc.tile_pool(name="pband", bufs=1, space="PSUM") as pb, \
             tc.tile_pool(name="pnp", bufs=2, space="PSUM") as pnp:

            wb = cp.tile([P, wcols], BF16)
            nc.gpsimd.dma_start(wb[:], wb_d[:])
            w32 = cp.tile([P, w32cols], F32)
            nc.gpsimd.dma_start(w32[:], w32_d[:])
            bbl = cp.tile([P, bcols], F32)
            nc.gpsimd.dma_start(bbl[:], bb_d[:])
            coffs = cp.tile([P, 2 * NPB], I32)
            nc.gpsimd.dma_start(coffs[:], coffs_d[:])
            hT = cp.tile([H + 1, SHP], BF16, tag="h")
            nc.gpsimd.dma_start(hT[H:H + 1, :], npads_d[:])
            ATd = cp.tile([P, SHP], BF16, tag="at")

            def W(name, rows=P):
                o, w = lay[name]
                return wb[0:rows, o:o + w]

            def W32(name, rows=P):
                o, w = lay32[name]
                return w32[0:rows, o:o + w]

            def B(name, rows=P):
                o, w = blay[name]
                return bbl[0:rows, o:o + w]

            def chunks(n, w=512):
                c = 0
                while c < n:
                    yield c, min(w, n - c)
                    c += w

            # ---- embed: hT = (nf_aug).T @ embW_aug  (bias folded) ----
            for c, w in chunks(SHP):
                nf = st.tile([ND + 1, 512], BF16, tag="nf")
                nc.sync.dma_start(nf[:, :w], nfT_d[:, c:c + w])
                ps = pnp.tile([P, 512], F32, tag="np")
                nc.tensor.matmul(ps[0:H, :w], W("embW", ND + 1), nf[:, :w],
                                 start=True, stop=True, skip_group_check=True)
                nc.scalar.copy(hT[0:H, c:c + w], ps[0:H, :w])

            # ---- layer-0 B table: local compute from global nf ----
            NB0 = TROWS // P
            GB = 8
            for g0 in range(0, NB0, GB):
                gn = min(GB, NB0 - g0)
                nfg = st.tile([ND + 1, GB * P], BF16, tag="nfg")
                nc.sync.dma_start(nfg[:, :gn * P],
                                  nfTg_d[:, g0 * P:(g0 + gn) * P])
                bst0 = st.tile([P, GB * H], BF16, tag="bst0")
                for g in range(gn):
                    psB = pnp.tile([P, 512], F32, tag="np")
                    nc.tensor.matmul(psB[:, 0:H],
                                     nfg[:, g * P:(g + 1) * P],
                                     W("nfW0", ND + 1),
                                     start=True, stop=True,
                                     skip_group_check=True)
                    nc.scalar.copy(bst0[:, g * H:(g + 1) * H], psB[:, 0:H])
                nc.sync.dma_start(
                    btabs[0][g0 * P:(g0 + gn) * P, :].rearrange(
                        "(c p) h -> p c h", p=P),
                    bst0[:, :gn * H].rearrange("p (c h) -> p c h", h=H))

            # ---- layer-0 A table ----
            for c, w in chunks(SHP):
                psA = pnp.tile([P, 512], F32, tag="np")
                nc.tensor.matmul(psA[:, :w], W("We1rd0", H), hT[0:H, c:c + w],
                                 start=True, stop=True, skip_group_check=True)
                nc.scalar.copy(ATd[:, c:c + w], psA[:, :w])

            if DEBUG:
                nc.gpsimd.dma_start(dbg["h0"][:], hT[0:H, :])
                for c, w in chunks(SHP):
                    tf = st.tile([P, 512], F32, tag="dbgf")
                    nc.vector.tensor_copy(tf[:, :w], ATd[:, c:c + w])
                    nc.gpsimd.dma_start(dbg["at0"][:, c:c + w], tf[:, :w])
                for g in range(TROWS // P):
                    tb = st.tile([P, H], BF16, tag="dbgb")
                    nc.gpsimd.dma_start(tb[:], btabs[0][g * P:(g + 1) * P, :])
                    tf = st.tile([P, H], F32, tag="dbgbf")
                    nc.vector.tensor_copy(tf[:], tb[:])
                    nc.gpsimd.dma_start(dbg["bt0"][g * P:(g + 1) * P, :],
                                        tf[:])

            pending_ags = []
            half_used = meta["half_used"]
            # stale bg blocks are read (and discarded via the poison path)
            # when a half's gather is skipped -- make sure no buffer ever
            # holds uninitialized SBUF.
            for _ in range(4):
                bgz = st.tile([P, BATCH * P], BF16, tag="bg")
                nc.vector.memset(bgz[:], 0.0)

            def emit_edge_batch(l, blo, p0, nb, t1):
                nw = nb * P
                bg = st.tile([P, BATCH * P], BF16, tag="bg")
                for i in range(nb):
                    for half in range(2):
                        if not half_used[p0 + i, half]:
                            continue
                        q = 2 * (p0 + i) + half
                        nc.gpsimd.indirect_dma_start(
                            out=bg[:, i * P + half * H:i * P + (half + 1) * H],
                            out_offset=None, in_=btabs[l][:],
                            in_offset=IndirectOffsetOnAxis(
                                ap=coffs[:, q:q + 1], axis=0))
                et = st.tile([EFR, BATCH * P], BF16, tag="eft")
                nc.sync.dma_start(et[:, :nw], eft_d[:, p0 * P:p0 * P + nw])
                pre = ppre.tile([P, BATCH * P], F32, tag="pre")
                nc.tensor.matmul(pre[:, :nw], W(f"Wefd{l}", EFR), et[:, :nw],
                                 start=True, stop=False, skip_group_check=True)
                for i in range(nb):
                    jb = pairs[p0 + i][1]
                    nc.tensor.matmul(pre[:, i * P:(i + 1) * P], W("I128"),
                                     ATd[:, jb:jb + P],
                                     start=False, stop=False,
                                     skip_group_check=True)
                for i in range(nb):
                    nc.tensor.matmul(pre[:, i * P:(i + 1) * P],
                                     bg[:, i * P:(i + 1) * P], W("I128"),
                                     start=False, stop=(i == nb - 1),
                                     skip_group_check=True)
                s1 = st.tile([P, BATCH * P], BF16, tag="s1")
                nc.scalar.activation(s1[:, :nw], pre[:, :nw], ACT.Silu,
                                     bias=B(f"be1st{l}"))
                z = pz.tile([P, BATCH * P], F32, tag="z")
                nc.tensor.matmul(z[:, :nw], W(f"BD{l}"), s1[:, :nw],
                                 start=True, stop=True, skip_group_check=True)
                m2 = st.tile([P, BATCH * P], BF16, tag="m2")
                nc.scalar.activation(m2[:, :nw], z[:, :nw], ACT.Silu,
                                     bias=B(f"be2st{l}"))
                if DEBUG and l == 0:
                    nc.gpsimd.dma_start(
                        dbg["m2"][:, p0 * P:p0 * P + nw], m2[:, :nw])
                    bgf = st.tile([P, BATCH * P], F32, tag="dbgbg")
                    nc.vector.tensor_copy(bgf[:, :nw], bg[:, :nw])
                    nc.gpsimd.dma_start(
                        dbg["bg"][:, p0 * P:p0 * P + nw], bgf[:, :nw])
                    prf = st.tile([P, BATCH * P], F32, tag="dbgpre")
                    nc.vector.tensor_copy(prf[:, :nw], pre[:, :nw])
                    nc.gpsimd.dma_start(
                        dbg["pre"][:, p0 * P:p0 * P + nw], prf[:, :nw])
                for i in range(nb):
                    jb = pairs[p0 + i][1]
                    nc.tensor.matmul(t1[:, jb - blo:jb - blo + P],
                                     W(f"Wn1a2{l}", P),
                                     m2[:, i * P:(i + 1) * P],
                                     start=False, stop=False,
                                     skip_group_check=True)

            # band -> pair ranges
            band_pairs = [[] for _ in range(NB)]
            for p, (b, _) in enumerate(pairs):
                band_pairs[b].append(p)

            for l in range(L):
                for b in range(NB):
                    blo, bn = band_base[b], band_rows[b]
                    # flush deferred AllGathers before this band's gathers;
                    # their input DMAs completed during the previous band's
                    # node phase, so the gpsimd stall here is tiny.
                    for ag in pending_ags:
                        ag()
                    pending_ags.clear()
                    t1 = pb.tile([H, BAND], F32, tag="t1")
                    for c, w in chunks(bn):
                        nc.tensor.matmul(t1[:, c:c + w], W(f"Wn1h{l}", H + 1),
                                         hT[0:H + 1, blo + c:blo + c + w],
                                         start=True, stop=False,
                                         skip_group_check=True)
                    plist = band_pairs[b]
                    for p0 in range(plist[0] if plist else 0,
                                    (plist[-1] + 1) if plist else 0, BATCH):
                        emit_edge_batch(l, blo, p0, BATCH, t1)
                    # node phase
                    rT = st.tile([H + 1, BAND], BF16, tag="rT")
                    nc.scalar.activation(rT[0:H, :bn], t1[:, :bn], ACT.Relu,
                                         bias=B(f"bn1{l}", H))
                    nc.vector.memset(rT[H:H + 1, :bn], 1.0)
                    if DEBUG and l == 0 and b == 0:
                        tf = st.tile([H, BAND], F32, tag="dbgr")
                        nc.vector.tensor_copy(tf[:, :bn], rT[0:H, :bn])
                        nc.gpsimd.dma_start(dbg["rt0"][:, :bn], tf[:, :bn])
                    for c, w in chunks(bn):
                        ps = pnp.tile([P, 512], F32, tag="np")
                        nc.tensor.matmul(ps[0:H, :w], W(f"Wn2{l}", H + 1),
                                         rT[:, c:c + w],
                                         start=True, stop=False,
                                         skip_group_check=True)
                        nc.tensor.matmul(ps[0:H, :w], W("I64", H),
                                         hT[0:H, blo + c:blo + c + w],
                                         start=False, stop=True,
                                         skip_group_check=True)
                        nc.scalar.copy(hT[0:H, blo + c:blo + c + w],
                                       ps[0:H, :w])
                    if l < L - 1:
                        # next-layer A + B for this band
                        for c, w in chunks(bn):
                            psA = pnp.tile([P, 512], F32, tag="np")
                            nc.tensor.matmul(psA[:, :w],
                                             W(f"We1rd{l + 1}", H),
                                             hT[0:H, blo + c:blo + c + w],
                                             start=True, stop=True,
                                             skip_group_check=True)
                            nc.scalar.copy(ATd[:, blo + c:blo + c + w],
                                           psA[:, :w])
                        nch = bn // P
                        bst = st.tile([P, (BAND // P) * H], BF16, tag="bst")
                        for c1 in range(nch):
                            psB = pnp.tile([P, 512], F32, tag="np")
                            nc.tensor.matmul(
                                psB[:, 0:H],
                                hT[0:H, blo + c1 * P:blo + (c1 + 1) * P],
                                W(f"We1c{l + 1}", H),
                                start=True, stop=True, skip_group_check=True)
                            nc.scalar.copy(bst[:, c1 * H:(c1 + 1) * H],
                                           psB[:, 0:H])
                        nc.sync.dma_start(
                            bshs[l][blo:blo + bn, :].rearrange(
                                "(p c) h -> p c h", c=nch),
                            bst[:, :nch * H].rearrange(
                                "p (c h) -> p c h", h=H))

                        def mk_ag(l=l, b=b, blo=blo, bn=bn):
                            def ag():
                                r0 = band_base[b] * NCORES
                                nc.gpsimd.collective_compute(
                                    "AllGather", mybir.AluOpType.bypass,
                                    replica_groups=[list(range(NCORES))],
                                    ins=[bshs[l][blo:blo + bn, :].opt()],
                                    outs=[btabs[l + 1][
                                        r0:r0 + NCORES * bn, :].opt()])
                            return ag
                        pending_ags.append(mk_ag())

                if DEBUG and l == 0:
                    nc.gpsimd.dma_start(dbg["h1"][:], hT[0:H, :])

            # flush any remaining AGs (last layer has none)
            for ag in pending_ags:
                ag()
            pending_ags.clear()

            # ---- unembed ----
            for c, w in chunks(SHP):
                ps = pnp.tile([P, 512], F32, tag="np")
                nc.tensor.matmul(ps[0:H, :w], W("Wu1", H), hT[0:H, c:c + w],
                                 start=True, stop=True, skip_group_check=True)
                sT = st.tile([H + 1, 512], BF16, tag="sT")
                nc.scalar.activation(sT[0:H, :w], ps[0:H, :w], ACT.Silu,
                                     bias=B("bu1", H))
                nc.vector.memset(sT[H:H + 1, :w], 1.0)
                ps2 = pnp.tile([P, 512], F32, tag="np")
                nc.tensor.matmul(ps2[0:OD, :w], W("Wu2", H + 1), sT[:, :w],
                                 start=True, stop=True, skip_group_check=True)
                ot = st.tile([OD, 512], F32, tag="ot")
                nc.scalar.copy(ot[:, :w], ps2[0:OD, :w])
                nc.sync.dma_start(out_d[:, c:c + w], ot[:, :w])

    return nc


# ---------------------------------------------------------------- entry

def kernel(node_features, edge_indices, edges_features, batch_size,
           emb_W, emb_b, We1, be1, We2, be2,
           Wn1, bn1, Wn2, bn2, Wu1, bu1, Wu2, bu2):
    node_features = np.ascontiguousarray(np.asarray(node_features, np.float32))
    edge_indices = np.ascontiguousarray(np.asarray(edge_indices)).astype(np.int64)
    edges_features = np.ascontiguousarray(np.asarray(edges_features, np.float32))
    fl = lambda x: np.asarray(x, np.float32)
    emb_W, emb_b = fl(emb_W), fl(emb_b)
    We1, be1, We2, be2 = fl(We1), fl(be1), fl(We2), fl(be2)
    Wn1, bn1, Wn2, bn2 = fl(Wn1), fl(bn1), fl(Wn2), fl(bn2)
    Wu1, bu1, Wu2, bu2 = fl(Wu1), fl(bu1), fl(Wu2), fl(bu2)

    N, ND = node_features.shape
    ED = edges_features.shape[1]
    L, _, H = We1.shape
    OD = Wu2.shape[1]

    meta = _prep(node_features, edge_indices, edges_features, We1)
    SH, SHP = meta["SH"], meta["SHP"]

    blob, bias, blob32 = _weights_blobs(emb_W, emb_b, We1, be1, We2, be2,
                                        Wn1, bn1, Wn2, bn2, Wu1, bu1,
                                        Wu2, bu2, H, ND, ED, OD)

    # global node-feature table in btab row order (for local B0 compute)
    TROWS = SHP * NCORES
    nfg = np.zeros((TROWS, ND + 1), np.float32)
    band_base, band_rows = meta["band_base"], meta["band_rows"]
    NB = meta["NB"]
    for s in range(NCORES):
        lo = s * SH
        nloc = min(SH, N - lo)
        nf_s = np.zeros((SHP, ND), np.float32)
        nf_s[:nloc] = node_features[lo:lo + nloc][meta["perms"][s]]
        for b in range(NB):
            bn = band_rows[b]
            nch = bn // P
            blk = nf_s[band_base[b]:band_base[b] + bn]       # [bn, ND]
            lane_major = blk.reshape(nch, P, ND).transpose(1, 0, 2) \
                            .reshape(bn, ND)
            r0 = band_base[b] * NCORES + s * bn
            nfg[r0:r0 + bn, :ND] = lane_major
    nfg[:, ND] = 1.0
    nfTg = np.ascontiguousarray(nfg.T).astype(BF)

    in_maps = []
    for s in range(NCORES):
        lo = s * SH
        nloc = min(SH, N - lo)
        nfT = np.zeros((ND + 1, SHP), np.float32)
        nfT[:ND, :nloc] = node_features[lo:lo + nloc][meta["perms"][s]].T
        nfT[ND, :] = 1.0
        in_maps.append({
            "nfT": nfT.astype(BF),
            "nfTg": nfTg,
            "eft": meta["efts"][s],
            "coffs": meta["coffs"][s],
            "wblob": blob,
            "bblob": bias,
            "w32": blob32,
            "npads": meta["npads"][s].astype(BF),
        })

    nc = _build(ND, ED, L, H, OD, meta, blob.shape[1], bias.shape[1],
                blob32.shape[1])
    _spread_swdge_queues(nc)
    _split_sync_waits(nc)
    res = run_bass_kernel_spmd(nc, in_maps, core_ids=list(range(NCORES)))
    out = np.zeros((N, OD), np.float32)
    for s in range(NCORES):
        predT = res.results[s]["out"]
        lo = s * SH
        nloc = min(SH, N - lo)
        out[lo + meta["perms"][s]] = predT[:, :nloc].T
    return out


# revision 9
# speedup vs baseline: 1.0010x; 1.0010x over previous
"""GNN message-passing kernel for 8 TRN2 NeuronCores (Bass/Tile), v2.

Design (edge-parallel by destination, per sharding hint):
  - Nodes sharded into 8 contiguous ranges; within each shard nodes are
    relabeled by descending local in-degree so the k-th incoming edge of
    every node forms a tile whose destinations are a prefix [0, n_k).
  - The edge MLP input concat is decomposed: pre = A[row] + B[col] + C with
    A = h@We1[:H], B = h@We1[H:2H], C = ef@We1[2H:].  A is read
    sequentially (prefix property), B is gathered per 128-edge tile via
    indirect DMA from a replicated bf16 HBM table, C is computed on-chip
    from a streamed bf16 edge-feature tensor (poison row zeroes pad edges
    through silu).
  - Tiles are processed in PAIRS (k, j),(k+1, j) stacked on partitions
    0-63 / 64-127; both halves target the same t1 node columns so one
    matmul per pair handles phi_edge L2 (block-diag We2), and one matmul
    per pair both applies Wn1[H:] per-edge and scatter-adds into the
    per-band PSUM accumulator t1 (segment-sum fused into the node MLP).
  - B tables are AllGathered per *band* so each layer's collective
    overlaps the previous layer's compute; layer 0's table is computed
    locally from node_features on every core (no collective).
  - Activations/weights in bf16 (tolerance 2e-2), PSUM accumulation fp32.
"""

import sys

if "/opt/trn_rl_repo" not in sys.path:
    sys.path.insert(0, "/opt/trn_rl_repo")

import numpy as np
import ml_dtypes

import concourse.bass as bass
import concourse.mybir as mybir
import concourse.tile as tile
from concourse.bass import IndirectOffsetOnAxis
from concourse.bass_utils import run_bass_kernel_spmd

NCORES = 8
P = 128
BATCH = 4          # pairs per edge batch (= 8 original tiles)
BAND = 1536        # nodes per t1 accumulation band (3 PSUM banks)
POISON = -100.0
DEBUG = False
F32 = mybir.dt.float32
BF16 = mybir.dt.bfloat16
I32 = mybir.dt.int32
ACT = mybir.ActivationFunctionType
BF = ml_dtypes.bfloat16


def _spread_swdge_queues(nc, nq=2):
    """indirect_dma_start pins queue="qPoolDynamic" (queue 0); alternate
    gathers across the allocated SWDGE queues so both GpSimd cores emit
    descriptors in parallel."""
    i = 0
    for func in nc.m.functions:
        for bb in func.blocks:
            for inst in bb.instructions:
                if (isinstance(inst, mybir.InstDMACopy)
                        and getattr(inst, "queue", None) == "qPoolDynamic"
                        and any(getattr(a, "dynamic_ap_info", None) is not None
                                for a in inst.ins + inst.outs)):
                    q = i % nq
                    if q:
                        inst.queue = f"qPoolDynamic{q}"
                    i += 1
    return i


def _split_sync_waits(nc):
    """Two walrus/HW quirks, one pass:
    1. this walrus build accepts only one sync-wait per instruction; move
       extras onto same-engine NOPs inserted just before.
    2. walrus lowers a self-loading InstMatmult into LDWEIGHTS+MATMUL with
       the sync-wait on the MATMUL only, so the stationary-operand load can
       race ahead of its producer; move ALL matmul waits onto NOPs so the
       sequencer stalls before LDWEIGHTS."""
    cnt = 0
    for func in nc.m.functions:
        for bb in func.blocks:
            out = []
            changed = False
            for inst in bb.instructions:
                si = inst.sync_info
                if si is not None and si.on_wait:
                    is_mm = isinstance(inst, mybir.InstMatmult)
                    if is_mm or len(si.on_wait) > 1:
                        extra = list(si.on_wait if is_mm else si.on_wait[:-1])
                        keep = None if is_mm else si.on_wait[-1]
                        del si.on_wait[:]
                        if keep is not None:
                            si.on_wait.append(keep)
                        for w in extra:
                            cnt += 1
                            nop = mybir.InstNoOp(name=f"WS-{cnt}", ins=[],
                                                 outs=[])
                            nop.engine = inst.engine
                            nop.sync_info = mybir.SyncInfo(on_wait=[w],
                                                           on_update=[])
                            out.append(nop)
                            changed = True
                out.append(inst)
            if changed:
                bb.instructions[:] = out
    return cnt


# ---------------------------------------------------------------- host prep

def _prep(node_features, edge_indices, edges_features, We1):
    N = node_features.shape[0]
    E = edge_indices.shape[1]
    ED = edges_features.shape[1]
    row = edge_indices[0].astype(np.int64)
    col = edge_indices[1].astype(np.int64)
    SH = -(-N // NCORES)
    SHP = -(-SH // P) * P
    NB = -(-SHP // BAND)
    band_rows = [min(BAND, SHP - b * BAND) for b in range(NB)]
    band_base = [b * BAND for b in range(NB)]

    # per-shard degree sort
    perms, inv_perms, deg_sorted = [], [], []
    core_edges = []
    for s in range(NCORES):
        lo, hi = s * SH, min((s + 1) * SH, N)
        eidx = np.nonzero((row >= lo) & (row < hi))[0]
        r_loc = row[eidx] - lo
        nloc = hi - lo
        deg = np.bincount(r_loc, minlength=nloc)
        perm = np.argsort(-deg, kind="stable")
        inv = np.empty(nloc, dtype=np.int64)
        inv[perm] = np.arange(nloc)
        slot = inv[r_loc]
        order = np.lexsort((col[eidx], slot))
        core_edges.append((slot[order], col[eidx][order], eidx[order]))
        perms.append(perm)
        inv_perms.append(inv)
        deg_sorted.append(deg[perm])

    maxdeg = max((int(d[0]) if len(d) else 0) for d in deg_sorted)
    n_k = [max(int((d > k).sum()) for d in deg_sorted) for k in range(maxdeg)]
    KP = -(-maxdeg // 2)

    # pair grid: band-major, then kp, then j; pad per band to BATCH
    pair_of = {}
    pairs = []          # (band, jb) per pair
    for b in range(NB):
        blo, bhi = band_base[b], band_base[b] + band_rows[b]
        for kp in range(KP):
            hi = min(n_k[2 * kp], bhi)
            j = blo
            while j < hi:
                pair_of[(kp, j)] = len(pairs)
                pairs.append((b, j))
                j += P
        while len(pairs) % BATCH:
            pairs.append((b, blo))          # dummy pair: all-pad
    NPB = len(pairs)

    # btab row index for a global node id: band-major, within a (band,
    # shard) block LANE-major (row = lane*nch + chunk) so the per-band
    # Bsh write is a single DMA from a [128, nch*H] SBUF tile.
    col_btab = np.empty(N, dtype=np.int64)
    br = np.array(band_rows)
    bb8 = np.array([band_base[bb] * NCORES for bb in range(NB)])
    nchs = br // P
    for s in range(NCORES):
        lo, hi = s * SH, min((s + 1) * SH, N)
        loc = inv_perms[s]                   # local node -> slot
        b2 = loc // BAND
        r2 = loc % BAND
        col_btab[lo:hi] = (bb8[b2] + s * br[b2]
                           + (r2 % P) * nchs[b2] + r2 // P)

    # per-shard streams
    efts, coffs_all = [], []
    for s in range(NCORES):
        slot, c_g, eidx = core_edges[s]
        first = np.searchsorted(slot, slot, side="left")
        rank = np.arange(len(slot)) - first
        keep = rank < maxdeg
        slot, c_g, eidx, rank = slot[keep], c_g[keep], eidx[keep], rank[keep]
        kp = rank // 2
        half = rank % 2
        jb = (slot // P) * P
        pidx = np.array([pair_of[(k, j)] for k, j in zip(kp, jb)],
                        dtype=np.int64)
        lane = slot % P

        eft = np.zeros((2 * (ED + 1), NPB * P), np.float32)
        eft[ED, :] = 1.0                     # pad indicator, half 0
        eft[2 * ED + 1, :] = 1.0             # pad indicator, half 1
        epos = pidx * P + lane
        # scatter: eft[half*(ED+1) + d, epos] = ef[eidx, d]; indicator -> 0
        ef_s = edges_features[eidx]
        for hval in (0, 1):
            m = half == hval
            base = hval * (ED + 1)
            eft[base + ED, epos[m]] = 0.0
            for d in range(ED):
                eft[base + d, epos[m]] = ef_s[m, d]

        co = np.zeros((P, 2 * NPB), np.int32)
        co[lane, 2 * pidx + half] = col_btab[c_g]
        efts.append(eft.astype(BF))
        coffs_all.append(co)

    # halves empty on ALL shards (SPMD shares one instruction stream):
    # their gathers fetch garbage that silu discards -> skip them.
    nk_per_s = [[int((deg_sorted[s] > k).sum()) for k in range(maxdeg)]
                for s in range(NCORES)]
    half_used = np.zeros((NPB, 2), bool)
    for p, (b, jb) in enumerate(pairs):
        kp_list = [k for (k, j2) in pair_of.items() if False]  # unused
    for (kp2, j2), p in pair_of.items():
        for h in range(2):
            k = 2 * kp2 + h
            if k < maxdeg and any(nk_per_s[s][k] > j2 for s in range(NCORES)):
                half_used[p, h] = True

    # pad-half count per slot (for the be2 pad-message correction): every
    # (pair, half) covering a slot that has no real edge contributes a
    # silu(be2) message; cancel it via the t1-init matmul.
    halves_per_block = np.zeros(SHP // P, np.int64)
    for (_, jb) in pairs:
        halves_per_block[jb // P] += 2
    npads = []
    for s in range(NCORES):
        d = np.zeros(SHP, np.int64)
        ds = deg_sorted[s]
        d[:len(ds)] = ds
        np_s = halves_per_block[(np.arange(SHP) // P)] - d
        npads.append((-np_s.astype(np.float32))[None, :])

    return dict(SH=SH, SHP=SHP, NB=NB, band_rows=band_rows,
                band_base=band_base, NPB=NPB, pairs=pairs, KP=KP,
                perms=perms, inv_perms=inv_perms, efts=efts,
                coffs=coffs_all, maxdeg=maxdeg, E=E, ED=ED, npads=npads,
                half_used=half_used)


def _blob_layout(L, H, ND, ED, OD):
    lay, o = {}, 0

    def add(name, w):
        nonlocal o
        lay[name] = (o, w)
        o += w

    add("I128", P)
    add("I64", H)
    add("embW", H)           # [ND+1, H] rows (emb_b folded)
    add("Wu1", H)
    add("Wu2", OD)           # [H+1, OD] rows (bu2 folded)
    for l in range(L):
        add(f"Wefd{l}", P)   # [2*(ED+1), 128] block-diag ef weights + poison
        add(f"BD{l}", P)     # [128, 128] block-diag We2
        add(f"Wn1a2{l}", H)  # [128, 64] stacked Wn1[H:]
        add(f"We1rd{l}", P)  # [64, 128] duplicated We1[:H]
        add(f"We1c{l}", H)   # [64, 64] We1[H:2H]
        add(f"Wn1h{l}", H)   # [64, 64] Wn1[:H]
        add(f"Wn2{l}", H)    # [65, 64] Wn2 + bn2 row
    add("nfW0", H)           # [ND+1, H] (embW+emb_b) @ We1c[0] for local B0
    lay["total"] = o
    return lay


def _bias_layout(L, H):
    lay, o = {}, 0

    def add(name, w):
        nonlocal o
        lay[name] = (o, w)
        o += w

    for l in range(L):
        add(f"be1st{l}", 1)   # [128, 1]
        add(f"be2st{l}", 1)   # [128, 1]
        add(f"bn1{l}", 1)     # [64, 1]
    add("bu1", 1)             # [64, 1]
    lay["total"] = o
    return lay


def _blob32_layout(L, H, ND, OD):
    """f32 weights for matmuls whose other operand is f32 (hT / m2 / nf)."""
    lay, o = {}, 0

    def add(name, w):
        nonlocal o
        lay[name] = (o, w)
        o += w

    add("I64", H)
    add("embW", H)            # [ND+1, H]
    add("Wu1", H)
    for l in range(L):
        add(f"Wn1h{l}", H)    # [65, 64]: row 64 = Wn1a.T @ silu(be2) (pad fix)
        add(f"We1rd{l}", P)   # [64, 128]
        add(f"We1c{l}", H)    # [64, 64]
        add(f"Wn1a2{l}", H)   # [128, 64]
    lay["total"] = o
    return lay


def _weights_blobs(emb_W, emb_b, We1, be1, We2, be2, Wn1, bn1, Wn2, bn2,
                   Wu1, bu1, Wu2, bu2, H, ND, ED, OD):
    L = We1.shape[0]
    lay = _blob_layout(L, H, ND, ED, OD)
    blob = np.zeros((P, lay["total"]), np.float32)

    def put(name, arr, prow=0):
        o, _ = lay[name]
        blob[prow:prow + arr.shape[0], o:o + arr.shape[1]] = arr

    put("I128", np.eye(P, dtype=np.float32))
    put("I64", np.eye(H, dtype=np.float32))
    put("embW", np.vstack([emb_W, emb_b[None, :]]))
    put("Wu1", Wu1)
    put("Wu2", np.vstack([Wu2, bu2[None, :]]))
    for l in range(L):
        wef = np.vstack([We1[l][2 * H:], np.full((1, H), POISON, np.float32)])
        wefd = np.zeros((2 * (ED + 1), P), np.float32)
        wefd[:ED + 1, :H] = wef
        wefd[ED + 1:, H:] = wef
        put(f"Wefd{l}", wefd)
        bd = np.zeros((P, P), np.float32)
        bd[:H, :H] = We2[l]
        bd[H:, H:] = We2[l]
        put(f"BD{l}", bd)
        put(f"Wn1a2{l}", np.vstack([Wn1[l][H:], Wn1[l][H:]]))
        put(f"We1rd{l}", np.hstack([We1[l][:H], We1[l][:H]]))
        put(f"We1c{l}", We1[l][H:2 * H])
        sb2 = be2[l] / (1.0 + np.exp(-be2[l]))
        put(f"Wn1h{l}", np.vstack([Wn1[l][:H], (Wn1[l][H:].T @ sb2)[None, :]]))
        put(f"Wn2{l}", np.vstack([Wn2[l], bn2[l][None, :]]))
    put("nfW0", np.vstack([emb_W, emb_b[None, :]]) @ We1[0][H:2 * H])

    blay = _bias_layout(L, H)
    bias = np.zeros((P, blay["total"]), np.float32)

    def putb(name, arr):
        o, _ = blay[name]
        bias[:arr.shape[0], o:o + 1] = arr[:, None]

    for l in range(L):
        putb(f"be1st{l}", np.concatenate([be1[l], be1[l]]))
        putb(f"be2st{l}", np.concatenate([be2[l], be2[l]]))
        putb(f"bn1{l}", bn1[l])
    putb("bu1", bu1)

    lay32 = _blob32_layout(L, H, ND, OD)
    blob32 = np.zeros((P, lay32["total"]), np.float32)

    def put32(name, arr):
        o, _ = lay32[name]
        blob32[:arr.shape[0], o:o + arr.shape[1]] = arr

    put32("I64", np.eye(H, dtype=np.float32))
    put32("embW", np.vstack([emb_W, emb_b[None, :]]))
    put32("Wu1", Wu1)
    for l in range(L):
        sb2 = be2[l] / (1.0 + np.exp(-be2[l]))       # silu(be2)
        corr = (Wn1[l][H:].T @ sb2)[None, :]          # pad-message row
        put32(f"Wn1h{l}", np.vstack([Wn1[l][:H], corr]))
        put32(f"We1rd{l}", np.hstack([We1[l][:H], We1[l][:H]]))
        put32(f"We1c{l}", We1[l][H:2 * H])
        put32(f"Wn1a2{l}", np.vstack([Wn1[l][H:], Wn1[l][H:]]))
    return blob.astype(BF), bias, blob32


# ---------------------------------------------------------------- builder

def _build(ND, ED, L, H, OD, meta, wcols, bcols, w32cols):
    lay = _blob_layout(L, H, ND, ED, OD)
    blay = _bias_layout(L, H)
    lay32 = _blob32_layout(L, H, ND, OD)
    SHP, NB = meta["SHP"], meta["NB"]
    band_rows, band_base = meta["band_rows"], meta["band_base"]
    NPB, pairs = meta["NPB"], meta["pairs"]
    EFR = 2 * (ED + 1)      # eft rows
    TROWS = SHP * NCORES    # btab rows

    nc = bass.Bass("TRN2", num_devices=NCORES, num_swdge_queues=2)
    nfT_d = nc.dram_tensor("nfT", [ND + 1, SHP], BF16, kind="ExternalInput")
    w32_d = nc.dram_tensor("w32", [P, w32cols], F32, kind="ExternalInput")
    dbg = {}
    if DEBUG:
        dbg["h0"] = nc.dram_tensor("dbg_h0", [H, SHP], F32,
                                   kind="ExternalOutput")
        dbg["at0"] = nc.dram_tensor("dbg_at0", [P, SHP], F32,
                                    kind="ExternalOutput")
        dbg["bt0"] = nc.dram_tensor("dbg_bt0", [SHP * NCORES, H], F32,
                                    kind="ExternalOutput")
        dbg["rt0"] = nc.dram_tensor("dbg_rt0", [H, BAND], F32,
                                    kind="ExternalOutput")
        dbg["h1"] = nc.dram_tensor("dbg_h1", [H, SHP], F32,
                                   kind="ExternalOutput")
        dbg["m2"] = nc.dram_tensor("dbg_m2", [P, NPB * P], F32,
                                   kind="ExternalOutput")
        dbg["pre"] = nc.dram_tensor("dbg_pre", [P, NPB * P], F32,
                                    kind="ExternalOutput")
        dbg["bg"] = nc.dram_tensor("dbg_bg", [P, NPB * P], F32,
                                   kind="ExternalOutput")
    nfTg_d = nc.dram_tensor("nfTg", [ND + 1, TROWS], BF16,
                            kind="ExternalInput")     # global, btab order
    eft_d = nc.dram_tensor("eft", [EFR, NPB * P], BF16, kind="ExternalInput")
    coffs_d = nc.dram_tensor("coffs", [P, 2 * NPB], I32, kind="ExternalInput")
    wb_d = nc.dram_tensor("wblob", [P, wcols], BF16, kind="ExternalInput")
    bb_d = nc.dram_tensor("bblob", [P, bcols], F32, kind="ExternalInput")
    npads_d = nc.dram_tensor("npads", [1, SHP], BF16,
                             kind="ExternalInput")
    out_d = nc.dram_tensor("out", [OD, SHP], F32, kind="ExternalOutput")
    btabs = [nc.dram_tensor(f"btab{l}", [TROWS, H], BF16, kind="Internal",
                            addr_space="Shared") for l in range(L)]
    bshs = [nc.dram_tensor(f"bsh{l}", [SHP, H], BF16, kind="Internal")
            for l in range(1, L)]

    with tile.TileContext(nc) as tc:
        with tc.tile_pool(name="const", bufs=1) as cp, \
             tc.tile_pool(name="st", bufs=6) as st, \
             tc.tile_pool(name="ppre", bufs=2, space="PSUM") as ppre, \
             tc.tile_pool(name="pz", bufs=1, space="PSUM") as pz, \
             tc.tile_pool(name="pband", bufs=1, space="PSUM") as pb, \
             tc.tile_pool(name="pnp", bufs=2, space="PSUM") as pnp:

            wb = cp.tile([P, wcols], BF16)
            nc.gpsimd.dma_start(wb[:], wb_d[:])
            w32 = cp.tile([P, w32cols], F32)
            nc.gpsimd.dma_start(w32[:], w32_d[:])
            bbl = cp.tile([P, bcols], F32)
            nc.gpsimd.dma_start(bbl[:], bb_d[:])
            coffs = cp.tile([P, 2 * NPB], I32)
            nc.gpsimd.dma_start(coffs[:], coffs_d[:])
            hT = cp.tile([H + 1, SHP], BF16, tag="h")
            nc.gpsimd.dma_start(hT[H:H + 1, :], npads_d[:])
            ATd = cp.tile([P, SHP], BF16, tag="at")

            def W(name, rows=P):
                o, w = lay[name]
                return wb[0:rows, o:o + w]

            def W32(name, rows=P):
                o, w = lay32[name]
                return w32[0:rows, o:o + w]

            def B(name, rows=P):
                o, w = blay[name]
                return bbl[0:rows, o:o + w]

            def chunks(n, w=512):
                c = 0
                while c < n:
                    yield c, min(w, n - c)
                    c += w

            # ---- embed: hT = (nf_aug).T @ embW_aug  (bias folded) ----
            for c, w in chunks(SHP):
                nf = st.tile([ND + 1, 512], BF16, tag="nf")
                nc.sync.dma_start(nf[:, :w], nfT_d[:, c:c + w])
                ps = pnp.tile([P, 512], F32, tag="np")
                nc.tensor.matmul(ps[0:H, :w], W("embW", ND + 1), nf[:, :w],
                                 start=True, stop=True, skip_group_check=True)
                nc.scalar.copy(hT[0:H, c:c + w], ps[0:H, :w])

            # ---- layer-0 B table: local compute from global nf ----
            NB0 = TROWS // P
            GB = 8
            for g0 in range(0, NB0, GB):
                gn = min(GB, NB0 - g0)
                nfg = st.tile([ND + 1, GB * P], BF16, tag="nfg")
                nc.sync.dma_start(nfg[:, :gn * P],
                                  nfTg_d[:, g0 * P:(g0 + gn) * P])
                bst0 = st.tile([P, GB * H], BF16, tag="bst0")
                for g in range(gn):
                    psB = pnp.tile([P, 512], F32, tag="np")
                    nc.tensor.matmul(psB[:, 0:H],
                                     nfg[:, g * P:(g + 1) * P],
                                     W("nfW0", ND + 1),
                                     start=True, stop=True,
                                     skip_group_check=True)
                    nc.scalar.copy(bst0[:, g * H:(g + 1) * H], psB[:, 0:H])
                nc.sync.dma_start(
                    btabs[0][g0 * P:(g0 + gn) * P, :].rearrange(
                        "(c p) h -> p c h", p=P),
                    bst0[:, :gn * H].rearrange("p (c h) -> p c h", h=H))

            # ---- layer-0 A table ----
            for c, w in chunks(SHP):
                psA = pnp.tile([P, 512], F32, tag="np")
                nc.tensor.matmul(psA[:, :w], W("We1rd0", H), hT[0:H, c:c + w],
                                 start=True, stop=True, skip_group_check=True)
                nc.scalar.copy(ATd[:, c:c + w], psA[:, :w])

            if DEBUG:
                nc.gpsimd.dma_start(dbg["h0"][:], hT[0:H, :])
                for c, w in chunks(SHP):
                    tf = st.tile([P, 512], F32, tag="dbgf")
                    nc.vector.tensor_copy(tf[:, :w], ATd[:, c:c + w])
                    nc.gpsimd.dma_start(dbg["at0"][:, c:c + w], tf[:, :w])
                for g in range(TROWS // P):
                    tb = st.tile([P, H], BF16, tag="dbgb")
                    nc.gpsimd.dma_start(tb[:], btabs[0][g * P:(g + 1) * P, :])
                    tf = st.tile([P, H], F32, tag="dbgbf")
                    nc.vector.tensor_copy(tf[:], tb[:])
                    nc.gpsimd.dma_start(dbg["bt0"][g * P:(g + 1) * P, :],
                                        tf[:])

            pending_ags = []
            half_used = meta["half_used"]
            # stale bg blocks are read (and discarded via the poison path)
            # when a half's gather is skipped -- make sure no buffer ever
            # holds uninitialized SBUF.
            for _ in range(6):
                bgz = st.tile([P, BATCH * P], BF16, tag="bg")
                nc.vector.memset(bgz[:], 0.0)

            def emit_edge_batch(l, blo, p0, nb, t1):
                nw = nb * P
                bg = st.tile([P, BATCH * P], BF16, tag="bg")
                for i in range(nb):
                    for half in range(2):
                        if not half_used[p0 + i, half]:
                            continue
                        q = 2 * (p0 + i) + half
                        nc.gpsimd.indirect_dma_start(
                            out=bg[:, i * P + half * H:i * P + (half + 1) * H],
                            out_offset=None, in_=btabs[l][:],
                            in_offset=IndirectOffsetOnAxis(
                                ap=coffs[:, q:q + 1], axis=0))
                et = st.tile([EFR, BATCH * P], BF16, tag="eft")
                nc.sync.dma_start(et[:, :nw], eft_d[:, p0 * P:p0 * P + nw])
                pre = ppre.tile([P, BATCH * P], F32, tag="pre")
                nc.tensor.matmul(pre[:, :nw], W(f"Wefd{l}", EFR), et[:, :nw],
                                 start=True, stop=False, skip_group_check=True)
                for i in range(nb):
                    jb = pairs[p0 + i][1]
                    nc.tensor.matmul(pre[:, i * P:(i + 1) * P], W("I128"),
                                     ATd[:, jb:jb + P],
                                     start=False, stop=False,
                                     skip_group_check=True)
                for i in range(nb):
                    nc.tensor.matmul(pre[:, i * P:(i + 1) * P],
                                     bg[:, i * P:(i + 1) * P], W("I128"),
                                     start=False, stop=(i == nb - 1),
                                     skip_group_check=True)
                s1 = st.tile([P, BATCH * P], BF16, tag="s1")
                nc.scalar.activation(s1[:, :nw], pre[:, :nw], ACT.Silu,
                                     bias=B(f"be1st{l}"))
                z = pz.tile([P, BATCH * P], F32, tag="z")
                nc.tensor.matmul(z[:, :nw], W(f"BD{l}"), s1[:, :nw],
                                 start=True, stop=True, skip_group_check=True)
                m2 = st.tile([P, BATCH * P], BF16, tag="m2")
                nc.scalar.activation(m2[:, :nw], z[:, :nw], ACT.Silu,
                                     bias=B(f"be2st{l}"))
                if DEBUG and l == 0:
                    nc.gpsimd.dma_start(
                        dbg["m2"][:, p0 * P:p0 * P + nw], m2[:, :nw])
                    bgf = st.tile([P, BATCH * P], F32, tag="dbgbg")
                    nc.vector.tensor_copy(bgf[:, :nw], bg[:, :nw])
                    nc.gpsimd.dma_start(
                        dbg["bg"][:, p0 * P:p0 * P + nw], bgf[:, :nw])
                    prf = st.tile([P, BATCH * P], F32, tag="dbgpre")
                    nc.vector.tensor_copy(prf[:, :nw], pre[:, :nw])
                    nc.gpsimd.dma_start(
                        dbg["pre"][:, p0 * P:p0 * P + nw], prf[:, :nw])
                for i in range(nb):
                    jb = pairs[p0 + i][1]
                    nc.tensor.matmul(t1[:, jb - blo:jb - blo + P],
                                     W(f"Wn1a2{l}", P),
                                     m2[:, i * P:(i + 1) * P],
                                     start=False, stop=False,
                                     skip_group_check=True)

            # band -> pair ranges
            band_pairs = [[] for _ in range(NB)]
            for p, (b, _) in enumerate(pairs):
                band_pairs[b].append(p)

            for l in range(L):
                for b in range(NB):
                    blo, bn = band_base[b], band_rows[b]
                    # flush deferred AllGathers before this band's gathers;
                    # their input DMAs completed during the previous band's
                    # node phase, so the gpsimd stall here is tiny.
                    for ag in pending_ags:
                        ag()
                    pending_ags.clear()
                    t1 = pb.tile([H, BAND], F32, tag="t1")
                    for c, w in chunks(bn):
                        nc.tensor.matmul(t1[:, c:c + w], W(f"Wn1h{l}", H + 1),
                                         hT[0:H + 1, blo + c:blo + c + w],
                                         start=True, stop=False,
                                         skip_group_check=True)
                    plist = band_pairs[b]
                    for p0 in range(plist[0] if plist else 0,
                                    (plist[-1] + 1) if plist else 0, BATCH):
                        emit_edge_batch(l, blo, p0, BATCH, t1)
                    # node phase
                    rT = st.tile([H + 1, BAND], BF16, tag="rT")
                    nc.scalar.activation(rT[0:H, :bn], t1[:, :bn], ACT.Relu,
                                         bias=B(f"bn1{l}", H))
                    nc.vector.memset(rT[H:H + 1, :bn], 1.0)
                    if DEBUG and l == 0 and b == 0:
                        tf = st.tile([H, BAND], F32, tag="dbgr")
                        nc.vector.tensor_copy(tf[:, :bn], rT[0:H, :bn])
                        nc.gpsimd.dma_start(dbg["rt0"][:, :bn], tf[:, :bn])
                    for c, w in chunks(bn):
                        ps = pnp.tile([P, 512], F32, tag="np")
                        nc.tensor.matmul(ps[0:H, :w], W(f"Wn2{l}", H + 1),
                                         rT[:, c:c + w],
                                         start=True, stop=False,
                                         skip_group_check=True)
                        nc.tensor.matmul(ps[0:H, :w], W("I64", H),
                                         hT[0:H, blo + c:blo + c + w],
                                         start=False, stop=True,
                                         skip_group_check=True)
                        nc.scalar.copy(hT[0:H, blo + c:blo + c + w],
                                       ps[0:H, :w])
                    if l < L - 1:
                        # next-layer A + B for this band
                        for c, w in chunks(bn):
                            psA = pnp.tile([P, 512], F32, tag="np")
                            nc.tensor.matmul(psA[:, :w],
                                             W(f"We1rd{l + 1}", H),
                                             hT[0:H, blo + c:blo + c + w],
                                             start=True, stop=True,
                                             skip_group_check=True)
                            nc.scalar.copy(ATd[:, blo + c:blo + c + w],
                                           psA[:, :w])
                        nch = bn // P
                        bst = st.tile([P, (BAND // P) * H], BF16, tag="bst")
                        for c1 in range(nch):
                            psB = pnp.tile([P, 512], F32, tag="np")
                            nc.tensor.matmul(
                                psB[:, 0:H],
                                hT[0:H, blo + c1 * P:blo + (c1 + 1) * P],
                                W(f"We1c{l + 1}", H),
                                start=True, stop=True, skip_group_check=True)
                            nc.scalar.copy(bst[:, c1 * H:(c1 + 1) * H],
                                           psB[:, 0:H])
                        nc.sync.dma_start(
                            bshs[l][blo:blo + bn, :].rearrange(
                                "(p c) h -> p c h", c=nch),
                            bst[:, :nch * H].rearrange(
                                "p (c h) -> p c h", h=H))

                        def mk_ag(l=l, b=b, blo=blo, bn=bn):
                            def ag():
                                r0 = band_base[b] * NCORES
                                nc.gpsimd.collective_compute(
                                    "AllGather", mybir.AluOpType.bypass,
                                    replica_groups=[list(range(NCORES))],
                                    ins=[bshs[l][blo:blo + bn, :].opt()],
                                    outs=[btabs[l + 1][
                                        r0:r0 + NCORES * bn, :].opt()])
                            return ag
                        pending_ags.append(mk_ag())

                if DEBUG and l == 0:
                    nc.gpsimd.dma_start(dbg["h1"][:], hT[0:H, :])

            # flush any remaining AGs (last layer has none)
            for ag in pending_ags:
                ag()
            pending_ags.clear()

            # ---- unembed ----
            for c, w in chunks(SHP):
                ps = pnp.tile([P, 512], F32, tag="np")
                nc.tensor.matmul(ps[0:H, :w], W("Wu1", H), hT[0:H, c:c + w],
                                 start=True, stop=True, skip_group_check=True)
                sT = st.tile([H + 1, 512], BF16, tag="sT")
                nc.scalar.activation(sT[0:H, :w], ps[0:H, :w], ACT.Silu,
                                     bias=B("bu1", H))
                nc.vector.memset(sT[H:H + 1, :w], 1.0)
                ps2 = pnp.tile([P, 512], F32, tag="np")
                nc.tensor.matmul(ps2[0:OD, :w], W("Wu2", H + 1), sT[:, :w],
                                 start=True, stop=True, skip_group_check=True)
                ot = st.tile([OD, 512], F32, tag="ot")
                nc.scalar.copy(ot[:, :w], ps2[0:OD, :w])
                nc.sync.dma_start(out_d[:, c:c + w], ot[:, :w])

    return nc


# ---------------------------------------------------------------- entry

def kernel(node_features, edge_indices, edges_features, batch_size,
           emb_W, emb_b, We1, be1, We2, be2,
           Wn1, bn1, Wn2, bn2, Wu1, bu1, Wu2, bu2):
    node_features = np.ascontiguousarray(np.asarray(node_features, np.float32))
    edge_indices = np.ascontiguousarray(np.asarray(edge_indices)).astype(np.int64)
    edges_features = np.ascontiguousarray(np.asarray(edges_features, np.float32))
    fl = lambda x: np.asarray(x, np.float32)
    emb_W, emb_b = fl(emb_W), fl(emb_b)
    We1, be1, We2, be2 = fl(We1), fl(be1), fl(We2), fl(be2)
    Wn1, bn1, Wn2, bn2 = fl(Wn1), fl(bn1), fl(Wn2), fl(bn2)
    Wu1, bu1, Wu2, bu2 = fl(Wu1), fl(bu1), fl(Wu2), fl(bu2)

    N, ND = node_features.shape
    ED = edges_features.shape[1]
    L, _, H = We1.shape
    OD = Wu2.shape[1]

    meta = _prep(node_features, edge_indices, edges_features, We1)
    SH, SHP = meta["SH"], meta["SHP"]

    blob, bias, blob32 = _weights_blobs(emb_W, emb_b, We1, be1, We2, be2,
                                        Wn1, bn1, Wn2, bn2, Wu1, bu1,
                                        Wu2, bu2, H, ND, ED, OD)

    # global node-feature table in btab row order (for local B0 compute)
    TROWS = SHP * NCORES
    nfg = np.zeros((TROWS, ND + 1), np.float32)
    band_base, band_rows = meta["band_base"], meta["band_rows"]
    NB = meta["NB"]
    for s in range(NCORES):
        lo = s * SH
        nloc = min(SH, N - lo)
        nf_s = np.zeros((SHP, ND), np.float32)
        nf_s[:nloc] = node_features[lo:lo + nloc][meta["perms"][s]]
        for b in range(NB):
            bn = band_rows[b]
            nch = bn // P
            blk = nf_s[band_base[b]:band_base[b] + bn]       # [bn, ND]
            lane_major = blk.reshape(nch, P, ND).transpose(1, 0, 2) \
                            .reshape(bn, ND)
            r0 = band_base[b] * NCORES + s * bn
            nfg[r0:r0 + bn, :ND] = lane_major
    nfg[:, ND] = 1.0
    nfTg = np.ascontiguousarray(nfg.T).astype(BF)

    in_maps = []
    for s in range(NCORES):
        lo = s * SH
        nloc = min(SH, N - lo)
        nfT = np.zeros((ND + 1, SHP), np.float32)
        nfT[:ND, :nloc] = node_features[lo:lo + nloc][meta["perms"][s]].T
        nfT[ND, :] = 1.0
        in_maps.append({
            "nfT": nfT.astype(BF),
            "nfTg": nfTg,
            "eft": meta["efts"][s],
            "coffs": meta["coffs"][s],
            "wblob": blob,
            "bblob": bias,
            "w32": blob32,
            "npads": meta["npads"][s].astype(BF),
        })

    nc = _build(ND, ED, L, H, OD, meta, blob.shape[1], bias.shape[1],
                blob32.shape[1])
    _spread_swdge_queues(nc)
    _split_sync_waits(nc)
    res = run_bass_kernel_spmd(nc, in_maps, core_ids=list(range(NCORES)))
    out = np.zeros((N, OD), np.float32)
    for s in range(NCORES):
        predT = res.results[s]["out"]
        lo = s * SH
        nloc = min(SH, N - lo)
        out[lo + meta["perms"][s]] = predT[:, :nloc].T
    return out


# revision 10
# speedup vs baseline: 1.1787x; 1.1775x over previous
"""GNN message-passing kernel for 8 TRN2 NeuronCores (Bass/Tile), v2.

Design (edge-parallel by destination, per sharding hint):
  - Nodes sharded into 8 contiguous ranges; within each shard nodes are
    relabeled by descending local in-degree so the k-th incoming edge of
    every node forms a tile whose destinations are a prefix [0, n_k).
  - The edge MLP input concat is decomposed: pre = A[row] + B[col] + C with
    A = h@We1[:H], B = h@We1[H:2H], C = ef@We1[2H:].  A is read
    sequentially (prefix property), B is gathered per 128-edge tile via
    indirect DMA from a replicated bf16 HBM table, C is computed on-chip
    from a streamed bf16 edge-feature tensor (poison row zeroes pad edges
    through silu).
  - Tiles are processed in PAIRS (k, j),(k+1, j) stacked on partitions
    0-63 / 64-127; both halves target the same t1 node columns so one
    matmul per pair handles phi_edge L2 (block-diag We2), and one matmul
    per pair both applies Wn1[H:] per-edge and scatter-adds into the
    per-band PSUM accumulator t1 (segment-sum fused into the node MLP).
  - B tables are AllGathered per *band* so each layer's collective
    overlaps the previous layer's compute; layer 0's table is computed
    locally from node_features on every core (no collective).
  - Activations/weights in bf16 (tolerance 2e-2), PSUM accumulation fp32.
"""

import sys

if "/opt/trn_rl_repo" not in sys.path:
    sys.path.insert(0, "/opt/trn_rl_repo")

import numpy as np
import ml_dtypes

import concourse.bass as bass
import concourse.mybir as mybir
import concourse.tile as tile
from concourse.bass import IndirectOffsetOnAxis
from concourse.bass_utils import run_bass_kernel_spmd

NCORES = 8
P = 128
BATCH = 4          # pairs per edge batch (= 8 original tiles)
BAND = 1536        # nodes per t1 accumulation band (3 PSUM banks)
POISON = -100.0
DEBUG = False
F32 = mybir.dt.float32
BF16 = mybir.dt.bfloat16
I32 = mybir.dt.int32
ACT = mybir.ActivationFunctionType
BF = ml_dtypes.bfloat16


def _spread_swdge_queues(nc, nq=2):
    """indirect_dma_start pins queue="qPoolDynamic" (queue 0); alternate
    gathers across the allocated SWDGE queues so both GpSimd cores emit
    descriptors in parallel."""
    i = 0
    for func in nc.m.functions:
        for bb in func.blocks:
            for inst in bb.instructions:
                if (isinstance(inst, mybir.InstDMACopy)
                        and getattr(inst, "queue", None) == "qPoolDynamic"
                        and any(getattr(a, "dynamic_ap_info", None) is not None
                                for a in inst.ins + inst.outs)):
                    q = i % nq
                    if q:
                        inst.queue = f"qPoolDynamic{q}"
                    i += 1
    return i


def _split_sync_waits(nc):
    """Two walrus/HW quirks, one pass:
    1. this walrus build accepts only one sync-wait per instruction; move
       extras onto same-engine NOPs inserted just before.
    2. walrus lowers a self-loading InstMatmult into LDWEIGHTS+MATMUL with
       the sync-wait on the MATMUL only, so the stationary-operand load can
       race ahead of its producer; move ALL matmul waits onto NOPs so the
       sequencer stalls before LDWEIGHTS."""
    cnt = 0
    for func in nc.m.functions:
        for bb in func.blocks:
            out = []
            changed = False
            for inst in bb.instructions:
                si = inst.sync_info
                if si is not None and si.on_wait:
                    is_mm = isinstance(inst, mybir.InstMatmult)
                    if is_mm or len(si.on_wait) > 1:
                        extra = list(si.on_wait if is_mm else si.on_wait[:-1])
                        keep = None if is_mm else si.on_wait[-1]
                        del si.on_wait[:]
                        if keep is not None:
                            si.on_wait.append(keep)
                        for w in extra:
                            cnt += 1
                            nop = mybir.InstNoOp(name=f"WS-{cnt}", ins=[],
                                                 outs=[])
                            nop.engine = inst.engine
                            nop.sync_info = mybir.SyncInfo(on_wait=[w],
                                                           on_update=[])
                            out.append(nop)
                            changed = True
                out.append(inst)
            if changed:
                bb.instructions[:] = out
    return cnt


# ---------------------------------------------------------------- host prep

def _prep(node_features, edge_indices, edges_features, We1):
    N = node_features.shape[0]
    E = edge_indices.shape[1]
    ED = edges_features.shape[1]
    row = edge_indices[0].astype(np.int64)
    col = edge_indices[1].astype(np.int64)
    SH = -(-N // NCORES)
    SHP = -(-SH // P) * P
    NB = -(-SHP // BAND)
    band_rows = [min(BAND, SHP - b * BAND) for b in range(NB)]
    band_base = [b * BAND for b in range(NB)]

    # per-shard degree sort
    perms, inv_perms, deg_sorted = [], [], []
    core_edges = []
    for s in range(NCORES):
        lo, hi = s * SH, min((s + 1) * SH, N)
        eidx = np.nonzero((row >= lo) & (row < hi))[0]
        r_loc = row[eidx] - lo
        nloc = hi - lo
        deg = np.bincount(r_loc, minlength=nloc)
        perm = np.argsort(-deg, kind="stable")
        inv = np.empty(nloc, dtype=np.int64)
        inv[perm] = np.arange(nloc)
        slot = inv[r_loc]
        order = np.lexsort((col[eidx], slot))
        core_edges.append((slot[order], col[eidx][order], eidx[order]))
        perms.append(perm)
        inv_perms.append(inv)
        deg_sorted.append(deg[perm])

    maxdeg = max((int(d[0]) if len(d) else 0) for d in deg_sorted)
    n_k = [max(int((d > k).sum()) for d in deg_sorted) for k in range(maxdeg)]
    KP = -(-maxdeg // 2)

    # pair grid: band-major, then kp, then j; pad per band to BATCH
    pair_of = {}
    pairs = []          # (band, jb) per pair
    for b in range(NB):
        blo, bhi = band_base[b], band_base[b] + band_rows[b]
        for kp in range(KP):
            hi = min(n_k[2 * kp], bhi)
            j = blo
            while j < hi:
                pair_of[(kp, j)] = len(pairs)
                pairs.append((b, j))
                j += P
        while len(pairs) % BATCH:
            pairs.append((b, blo))          # dummy pair: all-pad
    NPB = len(pairs)

    # btab row index for a global node id: band-major, within a (band,
    # shard) block LANE-major (row = lane*nch + chunk) so the per-band
    # Bsh write is a single DMA from a [128, nch*H] SBUF tile.
    col_btab = np.empty(N, dtype=np.int64)
    br = np.array(band_rows)
    bb8 = np.array([band_base[bb] * NCORES for bb in range(NB)])
    nchs = br // P
    for s in range(NCORES):
        lo, hi = s * SH, min((s + 1) * SH, N)
        loc = inv_perms[s]                   # local node -> slot
        b2 = loc // BAND
        r2 = loc % BAND
        col_btab[lo:hi] = (bb8[b2] + s * br[b2]
                           + (r2 % P) * nchs[b2] + r2 // P)

    # per-shard streams
    efts, coffs_all = [], []
    for s in range(NCORES):
        slot, c_g, eidx = core_edges[s]
        first = np.searchsorted(slot, slot, side="left")
        rank = np.arange(len(slot)) - first
        keep = rank < maxdeg
        slot, c_g, eidx, rank = slot[keep], c_g[keep], eidx[keep], rank[keep]
        kp = rank // 2
        half = rank % 2
        jb = (slot // P) * P
        pidx = np.array([pair_of[(k, j)] for k, j in zip(kp, jb)],
                        dtype=np.int64)
        lane = slot % P

        eft = np.zeros((2 * (ED + 1), NPB * P), np.float32)
        eft[ED, :] = 1.0                     # pad indicator, half 0
        eft[2 * ED + 1, :] = 1.0             # pad indicator, half 1
        epos = pidx * P + lane
        # scatter: eft[half*(ED+1) + d, epos] = ef[eidx, d]; indicator -> 0
        ef_s = edges_features[eidx]
        for hval in (0, 1):
            m = half == hval
            base = hval * (ED + 1)
            eft[base + ED, epos[m]] = 0.0
            for d in range(ED):
                eft[base + d, epos[m]] = ef_s[m, d]

        co = np.zeros((P, 2 * NPB), np.int32)
        co[lane, 2 * pidx + half] = col_btab[c_g]
        efts.append(eft.astype(BF))
        coffs_all.append(co)

    # halves empty on ALL shards (SPMD shares one instruction stream):
    # their gathers fetch garbage that silu discards -> skip them.
    nk_per_s = [[int((deg_sorted[s] > k).sum()) for k in range(maxdeg)]
                for s in range(NCORES)]
    half_used = np.zeros((NPB, 2), bool)
    for p, (b, jb) in enumerate(pairs):
        kp_list = [k for (k, j2) in pair_of.items() if False]  # unused
    for (kp2, j2), p in pair_of.items():
        for h in range(2):
            k = 2 * kp2 + h
            if k < maxdeg and any(nk_per_s[s][k] > j2 for s in range(NCORES)):
                half_used[p, h] = True

    # pad-half count per slot (for the be2 pad-message correction): every
    # (pair, half) covering a slot that has no real edge contributes a
    # silu(be2) message; cancel it via the t1-init matmul.
    halves_per_block = np.zeros(SHP // P, np.int64)
    for (_, jb) in pairs:
        halves_per_block[jb // P] += 2
    npads = []
    for s in range(NCORES):
        d = np.zeros(SHP, np.int64)
        ds = deg_sorted[s]
        d[:len(ds)] = ds
        np_s = halves_per_block[(np.arange(SHP) // P)] - d
        npads.append((-np_s.astype(np.float32))[None, :])

    return dict(SH=SH, SHP=SHP, NB=NB, band_rows=band_rows,
                band_base=band_base, NPB=NPB, pairs=pairs, KP=KP,
                perms=perms, inv_perms=inv_perms, efts=efts,
                coffs=coffs_all, maxdeg=maxdeg, E=E, ED=ED, npads=npads,
                half_used=half_used)


def _blob_layout(L, H, ND, ED, OD):
    lay, o = {}, 0

    def add(name, w):
        nonlocal o
        lay[name] = (o, w)
        o += w

    add("I128", P)
    add("I64", H)
    add("embW", H)           # [ND+1, H] rows (emb_b folded)
    add("Wu1", H)
    add("Wu2", OD)           # [H+1, OD] rows (bu2 folded)
    for l in range(L):
        add(f"Wefd{l}", P)   # [2*(ED+1), 128] block-diag ef weights + poison
        add(f"BD{l}", P)     # [128, 128] block-diag We2
        add(f"Wn1a2{l}", H)  # [128, 64] stacked Wn1[H:]
        add(f"We1rd{l}", P)  # [64, 128] duplicated We1[:H]
        add(f"We1c{l}", H)   # [64, 64] We1[H:2H]
        add(f"Wn1h{l}", H)   # [64, 64] Wn1[:H]
        add(f"Wn2{l}", H)    # [65, 64] Wn2 + bn2 row
    add("nfW0", H)           # [ND+1, H] (embW+emb_b) @ We1c[0] for local B0
    lay["total"] = o
    return lay


def _bias_layout(L, H):
    lay, o = {}, 0

    def add(name, w):
        nonlocal o
        lay[name] = (o, w)
        o += w

    for l in range(L):
        add(f"be1st{l}", 1)   # [128, 1]
        add(f"be2st{l}", 1)   # [128, 1]
        add(f"bn1{l}", 1)     # [64, 1]
    add("bu1", 1)             # [64, 1]
    lay["total"] = o
    return lay


def _blob32_layout(L, H, ND, OD):
    """f32 weights for matmuls whose other operand is f32 (hT / m2 / nf)."""
    lay, o = {}, 0

    def add(name, w):
        nonlocal o
        lay[name] = (o, w)
        o += w

    add("I64", H)
    add("embW", H)            # [ND+1, H]
    add("Wu1", H)
    for l in range(L):
        add(f"Wn1h{l}", H)    # [65, 64]: row 64 = Wn1a.T @ silu(be2) (pad fix)
        add(f"We1rd{l}", P)   # [64, 128]
        add(f"We1c{l}", H)    # [64, 64]
        add(f"Wn1a2{l}", H)   # [128, 64]
    lay["total"] = o
    return lay


def _weights_blobs(emb_W, emb_b, We1, be1, We2, be2, Wn1, bn1, Wn2, bn2,
                   Wu1, bu1, Wu2, bu2, H, ND, ED, OD):
    L = We1.shape[0]
    lay = _blob_layout(L, H, ND, ED, OD)
    blob = np.zeros((P, lay["total"]), np.float32)

    def put(name, arr, prow=0):
        o, _ = lay[name]
        blob[prow:prow + arr.shape[0], o:o + arr.shape[1]] = arr

    put("I128", np.eye(P, dtype=np.float32))
    put("I64", np.eye(H, dtype=np.float32))
    put("embW", np.vstack([emb_W, emb_b[None, :]]))
    put("Wu1", Wu1)
    put("Wu2", np.vstack([Wu2, bu2[None, :]]))
    for l in range(L):
        wef = np.vstack([We1[l][2 * H:], np.full((1, H), POISON, np.float32)])
        wefd = np.zeros((2 * (ED + 1), P), np.float32)
        wefd[:ED + 1, :H] = wef
        wefd[ED + 1:, H:] = wef
        put(f"Wefd{l}", wefd)
        bd = np.zeros((P, P), np.float32)
        bd[:H, :H] = We2[l]
        bd[H:, H:] = We2[l]
        put(f"BD{l}", bd)
        put(f"Wn1a2{l}", np.vstack([Wn1[l][H:], Wn1[l][H:]]))
        put(f"We1rd{l}", np.hstack([We1[l][:H], We1[l][:H]]))
        put(f"We1c{l}", We1[l][H:2 * H])
        sb2 = be2[l] / (1.0 + np.exp(-be2[l]))
        put(f"Wn1h{l}", np.vstack([Wn1[l][:H], (Wn1[l][H:].T @ sb2)[None, :]]))
        put(f"Wn2{l}", np.vstack([Wn2[l], bn2[l][None, :]]))
    put("nfW0", np.vstack([emb_W, emb_b[None, :]]) @ We1[0][H:2 * H])

    blay = _bias_layout(L, H)
    bias = np.zeros((P, blay["total"]), np.float32)

    def putb(name, arr):
        o, _ = blay[name]
        bias[:arr.shape[0], o:o + 1] = arr[:, None]

    for l in range(L):
        putb(f"be1st{l}", np.concatenate([be1[l], be1[l]]))
        putb(f"be2st{l}", np.concatenate([be2[l], be2[l]]))
        putb(f"bn1{l}", bn1[l])
    putb("bu1", bu1)

    lay32 = _blob32_layout(L, H, ND, OD)
    blob32 = np.zeros((P, lay32["total"]), np.float32)

    def put32(name, arr):
        o, _ = lay32[name]
        blob32[:arr.shape[0], o:o + arr.shape[1]] = arr

    put32("I64", np.eye(H, dtype=np.float32))
    put32("embW", np.vstack([emb_W, emb_b[None, :]]))
    put32("Wu1", Wu1)
    for l in range(L):
        sb2 = be2[l] / (1.0 + np.exp(-be2[l]))       # silu(be2)
        corr = (Wn1[l][H:].T @ sb2)[None, :]          # pad-message row
        put32(f"Wn1h{l}", np.vstack([Wn1[l][:H], corr]))
        put32(f"We1rd{l}", np.hstack([We1[l][:H], We1[l][:H]]))
        put32(f"We1c{l}", We1[l][H:2 * H])
        put32(f"Wn1a2{l}", np.vstack([Wn1[l][H:], Wn1[l][H:]]))
    return blob.astype(BF), bias, blob32


# ---------------------------------------------------------------- builder

def _build(ND, ED, L, H, OD, meta, wcols, bcols, w32cols):
    lay = _blob_layout(L, H, ND, ED, OD)
    blay = _bias_layout(L, H)
    lay32 = _blob32_layout(L, H, ND, OD)
    SHP, NB = meta["SHP"], meta["NB"]
    band_rows, band_base = meta["band_rows"], meta["band_base"]
    NPB, pairs = meta["NPB"], meta["pairs"]
    EFR = 2 * (ED + 1)      # eft rows
    TROWS = SHP * NCORES    # btab rows

    nc = bass.Bass("TRN2", num_devices=NCORES, num_swdge_queues=2)
    nfT_d = nc.dram_tensor("nfT", [ND + 1, SHP], BF16, kind="ExternalInput")
    w32_d = nc.dram_tensor("w32", [P, w32cols], F32, kind="ExternalInput")
    dbg = {}
    if DEBUG:
        dbg["h0"] = nc.dram_tensor("dbg_h0", [H, SHP], F32,
                                   kind="ExternalOutput")
        dbg["at0"] = nc.dram_tensor("dbg_at0", [P, SHP], F32,
                                    kind="ExternalOutput")
        dbg["bt0"] = nc.dram_tensor("dbg_bt0", [SHP * NCORES, H], F32,
                                    kind="ExternalOutput")
        dbg["rt0"] = nc.dram_tensor("dbg_rt0", [H, BAND], F32,
                                    kind="ExternalOutput")
        dbg["h1"] = nc.dram_tensor("dbg_h1", [H, SHP], F32,
                                   kind="ExternalOutput")
        dbg["m2"] = nc.dram_tensor("dbg_m2", [P, NPB * P], F32,
                                   kind="ExternalOutput")
        dbg["pre"] = nc.dram_tensor("dbg_pre", [P, NPB * P], F32,
                                    kind="ExternalOutput")
        dbg["bg"] = nc.dram_tensor("dbg_bg", [P, NPB * P], F32,
                                   kind="ExternalOutput")
    nfTg_d = nc.dram_tensor("nfTg", [ND + 1, TROWS], BF16,
                            kind="ExternalInput")     # global, btab order
    eft_d = nc.dram_tensor("eft", [EFR, NPB * P], BF16, kind="ExternalInput")
    coffs_d = nc.dram_tensor("coffs", [P, 2 * NPB], I32, kind="ExternalInput")
    wb_d = nc.dram_tensor("wblob", [P, wcols], BF16, kind="ExternalInput")
    bb_d = nc.dram_tensor("bblob", [P, bcols], F32, kind="ExternalInput")
    npads_d = nc.dram_tensor("npads", [1, SHP], BF16,
                             kind="ExternalInput")
    out_d = nc.dram_tensor("out", [OD, SHP], F32, kind="ExternalOutput")
    btabs = [nc.dram_tensor(f"btab{l}", [TROWS, H], BF16, kind="Internal",
                            addr_space="Shared") for l in range(L)]
    bshs = [nc.dram_tensor(f"bsh{l}", [SHP, H], BF16, kind="Internal")
            for l in range(1, L)]

    with tile.TileContext(nc) as tc:
        with tc.tile_pool(name="const", bufs=1) as cp, \
             tc.tile_pool(name="st", bufs=4) as st, \
             tc.tile_pool(name="ppre", bufs=2, space="PSUM") as ppre, \
             tc.tile_pool(name="pz", bufs=1, space="PSUM") as pz, \
             tc.tile_pool(name="pband", bufs=1, space="PSUM") as pb, \
             tc.tile_pool(name="pnp", bufs=2, space="PSUM") as pnp:

            wb = cp.tile([P, wcols], BF16)
            nc.gpsimd.dma_start(wb[:], wb_d[:])
            w32 = cp.tile([P, w32cols], F32)
            nc.gpsimd.dma_start(w32[:], w32_d[:])
            bbl = cp.tile([P, bcols], F32)
            nc.gpsimd.dma_start(bbl[:], bb_d[:])
            coffs = cp.tile([P, 2 * NPB], I32)
            nc.gpsimd.dma_start(coffs[:], coffs_d[:])
            hT = cp.tile([H + 1, SHP], BF16, tag="h")
            nc.gpsimd.dma_start(hT[H:H + 1, :], npads_d[:])
            ATd = cp.tile([P, SHP], BF16, tag="at")

            def W(name, rows=P):
                o, w = lay[name]
                return wb[0:rows, o:o + w]

            def W32(name, rows=P):
                o, w = lay32[name]
                return w32[0:rows, o:o + w]

            def B(name, rows=P):
                o, w = blay[name]
                return bbl[0:rows, o:o + w]

            def chunks(n, w=512):
                c = 0
                while c < n:
                    yield c, min(w, n - c)
                    c += w

            # ---- embed: hT = (nf_aug).T @ embW_aug  (bias folded) ----
            for c, w in chunks(SHP):
                nf = st.tile([ND + 1, 512], BF16, tag="nf")
                nc.sync.dma_start(nf[:, :w], nfT_d[:, c:c + w])
                ps = pnp.tile([P, 512], F32, tag="np")
                nc.tensor.matmul(ps[0:H, :w], W("embW", ND + 1), nf[:, :w],
                                 start=True, stop=True, skip_group_check=True)
                nc.scalar.copy(hT[0:H, c:c + w], ps[0:H, :w])

            # ---- layer-0 B table: local compute from global nf ----
            NB0 = TROWS // P
            GB = 8
            for g0 in range(0, NB0, GB):
                gn = min(GB, NB0 - g0)
                nfg = st.tile([ND + 1, GB * P], BF16, tag="nfg")
                nc.sync.dma_start(nfg[:, :gn * P],
                                  nfTg_d[:, g0 * P:(g0 + gn) * P])
                bst0 = st.tile([P, GB * H], BF16, tag="bst0")
                for g in range(gn):
                    psB = pnp.tile([P, 512], F32, tag="np")
                    nc.tensor.matmul(psB[:, 0:H],
                                     nfg[:, g * P:(g + 1) * P],
                                     W("nfW0", ND + 1),
                                     start=True, stop=True,
                                     skip_group_check=True)
                    nc.scalar.copy(bst0[:, g * H:(g + 1) * H], psB[:, 0:H])
                nc.sync.dma_start(
                    btabs[0][g0 * P:(g0 + gn) * P, :].rearrange(
                        "(c p) h -> p c h", p=P),
                    bst0[:, :gn * H].rearrange("p (c h) -> p c h", h=H))

            # ---- layer-0 A table ----
            for c, w in chunks(SHP):
                psA = pnp.tile([P, 512], F32, tag="np")
                nc.tensor.matmul(psA[:, :w], W("We1rd0", H), hT[0:H, c:c + w],
                                 start=True, stop=True, skip_group_check=True)
                nc.scalar.copy(ATd[:, c:c + w], psA[:, :w])

            if DEBUG:
                nc.gpsimd.dma_start(dbg["h0"][:], hT[0:H, :])
                for c, w in chunks(SHP):
                    tf = st.tile([P, 512], F32, tag="dbgf")
                    nc.vector.tensor_copy(tf[:, :w], ATd[:, c:c + w])
                    nc.gpsimd.dma_start(dbg["at0"][:, c:c + w], tf[:, :w])
                for g in range(TROWS // P):
                    tb = st.tile([P, H], BF16, tag="dbgb")
                    nc.gpsimd.dma_start(tb[:], btabs[0][g * P:(g + 1) * P, :])
                    tf = st.tile([P, H], F32, tag="dbgbf")
                    nc.vector.tensor_copy(tf[:], tb[:])
                    nc.gpsimd.dma_start(dbg["bt0"][g * P:(g + 1) * P, :],
                                        tf[:])

            pending_ags = []
            half_used = meta["half_used"]
            # stale bg blocks are read (and discarded via the poison path)
            # when a half's gather is skipped -- make sure no buffer ever
            # holds uninitialized SBUF.
            for _ in range(4):
                bgz = st.tile([P, BATCH * P], BF16, tag="bg")
                nc.vector.memset(bgz[:], 0.0)

            def emit_edge_batch(l, blo, p0, nb, t1):
                nw = nb * P
                bg = st.tile([P, BATCH * P], BF16, tag="bg")
                for i in range(nb):
                    for half in range(2):
                        if not half_used[p0 + i, half]:
                            continue
                        q = 2 * (p0 + i) + half
                        nc.gpsimd.indirect_dma_start(
                            out=bg[:, i * P + half * H:i * P + (half + 1) * H],
                            out_offset=None, in_=btabs[l][:],
                            in_offset=IndirectOffsetOnAxis(
                                ap=coffs[:, q:q + 1], axis=0))
                et = st.tile([EFR, BATCH * P], BF16, tag="eft")
                nc.sync.dma_start(et[:, :nw], eft_d[:, p0 * P:p0 * P + nw])
                pre = ppre.tile([P, BATCH * P], F32, tag="pre")
                nc.tensor.matmul(pre[:, :nw], W(f"Wefd{l}", EFR), et[:, :nw],
                                 start=True, stop=False, skip_group_check=True)
                for i in range(nb):
                    jb = pairs[p0 + i][1]
                    nc.tensor.matmul(pre[:, i * P:(i + 1) * P], W("I128"),
                                     ATd[:, jb:jb + P],
                                     start=False, stop=False,
                                     skip_group_check=True)
                for i in range(nb):
                    nc.tensor.matmul(pre[:, i * P:(i + 1) * P],
                                     bg[:, i * P:(i + 1) * P], W("I128"),
                                     start=False, stop=(i == nb - 1),
                                     skip_group_check=True)
                s1 = st.tile([P, BATCH * P], BF16, tag="s1")
                nc.scalar.activation(s1[:, :nw], pre[:, :nw], ACT.Silu,
                                     bias=B(f"be1st{l}"))
                z = pz.tile([P, BATCH * P], F32, tag="z")
                nc.tensor.matmul(z[:, :nw], W(f"BD{l}"), s1[:, :nw],
                                 start=True, stop=True, skip_group_check=True)
                m2 = st.tile([P, BATCH * P], BF16, tag="m2")
                nc.scalar.activation(m2[:, :nw], z[:, :nw], ACT.Silu,
                                     bias=B(f"be2st{l}"))
                if DEBUG and l == 0:
                    nc.gpsimd.dma_start(
                        dbg["m2"][:, p0 * P:p0 * P + nw], m2[:, :nw])
                    bgf = st.tile([P, BATCH * P], F32, tag="dbgbg")
                    nc.vector.tensor_copy(bgf[:, :nw], bg[:, :nw])
                    nc.gpsimd.dma_start(
                        dbg["bg"][:, p0 * P:p0 * P + nw], bgf[:, :nw])
                    prf = st.tile([P, BATCH * P], F32, tag="dbgpre")
                    nc.vector.tensor_copy(prf[:, :nw], pre[:, :nw])
                    nc.gpsimd.dma_start(
                        dbg["pre"][:, p0 * P:p0 * P + nw], prf[:, :nw])
                for i in range(nb):
                    jb = pairs[p0 + i][1]
                    nc.tensor.matmul(t1[:, jb - blo:jb - blo + P],
                                     W(f"Wn1a2{l}", P),
                                     m2[:, i * P:(i + 1) * P],
                                     start=False, stop=False,
                                     skip_group_check=True)

            # band -> pair ranges
            band_pairs = [[] for _ in range(NB)]
            for p, (b, _) in enumerate(pairs):
                band_pairs[b].append(p)

            for l in range(L):
                for b in range(NB):
                    blo, bn = band_base[b], band_rows[b]
                    # flush deferred AllGathers before this band's gathers;
                    # their input DMAs completed during the previous band's
                    # node phase, so the gpsimd stall here is tiny.
                    for ag in pending_ags:
                        ag()
                    pending_ags.clear()
                    t1 = pb.tile([H, BAND], F32, tag="t1")
                    for c, w in chunks(bn):
                        nc.tensor.matmul(t1[:, c:c + w], W(f"Wn1h{l}", H + 1),
                                         hT[0:H + 1, blo + c:blo + c + w],
                                         start=True, stop=False,
                                         skip_group_check=True)
                    plist = band_pairs[b]
                    for p0 in range(plist[0] if plist else 0,
                                    (plist[-1] + 1) if plist else 0, BATCH):
                        emit_edge_batch(l, blo, p0, BATCH, t1)
                    # node phase
                    rT = st.tile([H + 1, BAND], BF16, tag="rT")
                    nc.scalar.activation(rT[0:H, :bn], t1[:, :bn], ACT.Relu,
                                         bias=B(f"bn1{l}", H))
                    nc.vector.memset(rT[H:H + 1, :bn], 1.0)
                    if DEBUG and l == 0 and b == 0:
                        tf = st.tile([H, BAND], F32, tag="dbgr")
                        nc.vector.tensor_copy(tf[:, :bn], rT[0:H, :bn])
                        nc.gpsimd.dma_start(dbg["rt0"][:, :bn], tf[:, :bn])
                    for c, w in chunks(bn):
                        ps = pnp.tile([P, 512], F32, tag="np")
                        nc.tensor.matmul(ps[0:H, :w], W(f"Wn2{l}", H + 1),
                                         rT[:, c:c + w],
                                         start=True, stop=False,
                                         skip_group_check=True)
                        nc.tensor.matmul(ps[0:H, :w], W("I64", H),
                                         hT[0:H, blo + c:blo + c + w],
                                         start=False, stop=True,
                                         skip_group_check=True)
                        nc.scalar.copy(hT[0:H, blo + c:blo + c + w],
                                       ps[0:H, :w])
                    if l < L - 1:
                        # next-layer A + B for this band
                        for c, w in chunks(bn):
                            psA = pnp.tile([P, 512], F32, tag="np")
                            nc.tensor.matmul(psA[:, :w],
                                             W(f"We1rd{l + 1}", H),
                                             hT[0:H, blo + c:blo + c + w],
                                             start=True, stop=True,
                                             skip_group_check=True)
                            nc.scalar.copy(ATd[:, blo + c:blo + c + w],
                                           psA[:, :w])
                        nch = bn // P
                        bst = st.tile([P, (BAND // P) * H], BF16, tag="bst")
                        for c1 in range(nch):
                            psB = pnp.tile([P, 512], F32, tag="np")
                            nc.tensor.matmul(
                                psB[:, 0:H],
                                hT[0:H, blo + c1 * P:blo + (c1 + 1) * P],
                                W(f"We1c{l + 1}", H),
                                start=True, stop=True, skip_group_check=True)
                            nc.scalar.copy(bst[:, c1 * H:(c1 + 1) * H],
                                           psB[:, 0:H])
                        nc.sync.dma_start(
                            bshs[l][blo:blo + bn, :].rearrange(
                                "(p c) h -> p c h", c=nch),
                            bst[:, :nch * H].rearrange(
                                "p (c h) -> p c h", h=H))

                        def mk_ag(l=l, b=b, blo=blo, bn=bn):
                            def ag():
                                r0 = band_base[b] * NCORES
                                nc.gpsimd.collective_compute(
                                    "AllGather", mybir.AluOpType.bypass,
                                    replica_groups=[list(range(NCORES))],
                                    ins=[bshs[l][blo:blo + bn, :].opt()],
                                    outs=[btabs[l + 1][
                                        r0:r0 + NCORES * bn, :].opt()])
                            return ag
                        pending_ags.append(mk_ag())

                if DEBUG and l == 0:
                    nc.gpsimd.dma_start(dbg["h1"][:], hT[0:H, :])

            # flush any remaining AGs (last layer has none)
            for ag in pending_ags:
                ag()
            pending_ags.clear()

            # ---- unembed ----
            for c, w in chunks(SHP):
                ps = pnp.tile([P, 512], F32, tag="np")
                nc.tensor.matmul(ps[0:H, :w], W("Wu1", H), hT[0:H, c:c + w],
                                 start=True, stop=True, skip_group_check=True)
                sT = st.tile([H + 1, 512], BF16, tag="sT")
                nc.scalar.activation(sT[0:H, :w], ps[0:H, :w], ACT.Silu,
                                     bias=B("bu1", H))
                nc.vector.memset(sT[H:H + 1, :w], 1.0)
                ps2 = pnp.tile([P, 512], F32, tag="np")
                nc.tensor.matmul(ps2[0:OD, :w], W("Wu2", H + 1), sT[:, :w],
                                 start=True, stop=True, skip_group_check=True)
                ot = st.tile([OD, 512], F32, tag="ot")
                nc.scalar.copy(ot[:, :w], ps2[0:OD, :w])
                nc.sync.dma_start(out_d[:, c:c + w], ot[:, :w])

    return nc


# ---------------------------------------------------------------- entry

def kernel(node_features, edge_indices, edges_features, batch_size,
           emb_W, emb_b, We1, be1, We2, be2,
           Wn1, bn1, Wn2, bn2, Wu1, bu1, Wu2, bu2):
    node_features = np.ascontiguousarray(np.asarray(node_features, np.float32))
    edge_indices = np.ascontiguousarray(np.asarray(edge_indices)).astype(np.int64)
    edges_features = np.ascontiguousarray(np.asarray(edges_features, np.float32))
    fl = lambda x: np.asarray(x, np.float32)
    emb_W, emb_b = fl(emb_W), fl(emb_b)
    We1, be1, We2, be2 = fl(We1), fl(be1), fl(We2), fl(be2)
    Wn1, bn1, Wn2, bn2 = fl(Wn1), fl(bn1), fl(Wn2), fl(bn2)
    Wu1, bu1, Wu2, bu2 = fl(Wu1), fl(bu1), fl(Wu2), fl(bu2)

    N, ND = node_features.shape
    ED = edges_features.shape[1]
    L, _, H = We1.shape
    OD = Wu2.shape[1]

    meta = _prep(node_features, edge_indices, edges_features, We1)
    SH, SHP = meta["SH"], meta["SHP"]

    blob, bias, blob32 = _weights_blobs(emb_W, emb_b, We1, be1, We2, be2,
                                        Wn1, bn1, Wn2, bn2, Wu1, bu1,
                                        Wu2, bu2, H, ND, ED, OD)

    # global node-feature table in btab row order (for local B0 compute)
    TROWS = SHP * NCORES
    nfg = np.zeros((TROWS, ND + 1), np.float32)
    band_base, band_rows = meta["band_base"], meta["band_rows"]
    NB = meta["NB"]
    for s in range(NCORES):
        lo = s * SH
        nloc = min(SH, N - lo)
        nf_s = np.zeros((SHP, ND), np.float32)
        nf_s[:nloc] = node_features[lo:lo + nloc][meta["perms"][s]]
        for b in range(NB):
            bn = band_rows[b]
            nch = bn // P
            blk = nf_s[band_base[b]:band_base[b] + bn]       # [bn, ND]
            lane_major = blk.reshape(nch, P, ND).transpose(1, 0, 2) \
                            .reshape(bn, ND)
            r0 = band_base[b] * NCORES + s * bn
            nfg[r0:r0 + bn, :ND] = lane_major
    nfg[:, ND] = 1.0
    nfTg = np.ascontiguousarray(nfg.T).astype(BF)

    in_maps = []
    for s in range(NCORES):
        lo = s * SH
        nloc = min(SH, N - lo)
        nfT = np.zeros((ND + 1, SHP), np.float32)
        nfT[:ND, :nloc] = node_features[lo:lo + nloc][meta["perms"][s]].T
        nfT[ND, :] = 1.0
        in_maps.append({
            "nfT": nfT.astype(BF),
            "nfTg": nfTg,
            "eft": meta["efts"][s],
            "coffs": meta["coffs"][s],
            "wblob": blob,
            "bblob": bias,
            "w32": blob32,
            "npads": meta["npads"][s].astype(BF),
        })

    nc = _build(ND, ED, L, H, OD, meta, blob.shape[1], bias.shape[1],
                blob32.shape[1])
    _spread_swdge_queues(nc)
    _split_sync_waits(nc)
    res = run_bass_kernel_spmd(nc, in_maps, core_ids=list(range(NCORES)))
    out = np.zeros((N, OD), np.float32)
    for s in range(NCORES):
        predT = res.results[s]["out"]
        lo = s * SH
        nloc = min(SH, N - lo)
        out[lo + meta["perms"][s]] = predT[:, :nloc].T
    return out
